# revision 1
# baseline (speedup 1.0000x reference)
"""Two-layer GAT (PyG-style GATConv) on 8 Trainium2 NeuronCores.

Sharding: dst-nodes 1024/core; each core owns all edges into its dst range
(host-side graph partitioning: edges grouped into 64-dst blocks, padded to
128-edge tiles). GAT weights replicated.

v2 layout:
- a1 attention projections sharded: each core computes a1 for its 1024
  nodes from a host-folded w1a = combine(W1, att1) basis, AllGathers the
  [8192, 16] table (was: replicated full-N compute per core).
- Layer 1 aggregates input X with alpha folded into one-hot dst-selection
  matrices (segment softmax + scatter-add as TensorE matmuls); dst-side
  scores come from a one-hot broadcast matmul against the core's local a1
  rows (no per-edge dst gather).
- Layer 2: h2 = x2 @ W2 locally; h2 rows carry [h2 | a_src2 | a_dst2 | 1]
  so a single AllGather moves features + scores; the edge phase uses the
  alpha-scaled one-hot as matmul weights streaming h2 rows, with a
  ones-column accumulating the softmax denominator, normalized per
  dst-partition at block end. Output lands node-major (no transposes).
- Masked fc/classifier on the owning core; host reassembles the output.
"""
import numpy as np
from concourse import bass, mybir
import concourse.tile as tile
from concourse.bass_utils import run_bass_kernel_spmd
from concourse.vector_clock import ScopedClock, VectorClock
from concourse.masks import make_identity

N, E, M = 8192, 32768, 1024
D = 768
F = 6
H1 = 8
NCORES = 8
NSH = N // NCORES
DB = 64
NB = NSH // DB
P = 128
HW = D + 4          # h2ag row: 768 h2 | a_s2 | a_d2 | 1.0 | pad

f32 = mybir.dt.float32
bf16 = mybir.dt.bfloat16
fp8 = mybir.dt.float8e4
i32 = mybir.dt.int32
BF = mybir.dt.np(bf16)
F8 = mybir.dt.np(fp8)
FP8_S4 = True    # quantize aggT/W1 to fp8, DoubleRow matmuls in S4
FP8_S5 = False    # quantize x2/W2 to fp8, DoubleRow matmuls in S5
WSCALE = 32.0    # fp8 weight pre-scale (undone via activation scale)

NEG_SLOPE = 0.2

# test-harness knobs (harness calls kernel() with defaults: no tracing)
TRACE = False
TRACE_DIR = None
LAST = None


# ---------------------------------------------------------------------------
# The walrus build in this container rejects a Drain instruction with more
# than one semaphore wait ("Too many sync wait commands"); the default
# TileContext kernel-tail drain has many. Emit one single-wait drain per
# logical processor instead.
def _split_drain_and_barrier(self, tick_clock, wait_clock):
    gc = tick_clock.global_clock
    nprocs = 27
    for i in range(nprocs):
        mask = [0] * nprocs
        mask[i] = 1 << 30
        part = gc.elementwise_min(VectorClock(mask))
        d = self.nc.sync.drain()
        wait_clock.add_sem_waits(d.ins, ScopedClock({None: part}))
    self.nc.all_engine_barrier()
    popped = self.nc._tile_sem_poison_stack.pop()
    assert popped is self._sem_poison
    self.nc.clear_and_free_semaphores(list(self.sems.allocated().values()))
    self.nc.all_engine_barrier()


tile.TileContext._drain_and_barrier = _split_drain_and_barrier

MAX_WAITS = 1  # this walrus build rejects multi-sem-wait instructions


def _split_excess_waits(nc):
    """Move excess semaphore waits onto preceding same-engine NoOps."""
    n_split = 0
    for bb in nc.m.functions[0].blocks:
        insts = bb.instructions
        idx = 0
        while idx < len(insts):
            inst = insts[idx]
            si = inst.sync_info
            if si is not None and len(si.on_wait) > MAX_WAITS:
                waits = list(si.on_wait)
                keep = waits[-MAX_WAITS:]
                extra = waits[:-MAX_WAITS]
                for gi in range(0, len(extra), MAX_WAITS):
                    nop = mybir.InstNoOp(
                        name=f"WSPLIT-{nc.next_id()}",
                        sync_info=mybir.SyncInfo(
                            on_wait=extra[gi:gi + MAX_WAITS], on_update=[]),
                        bass_nofuse=True,
                        engine=inst.engine,
                        ins=[], outs=[],
                    )
                    nc.register_instruction(nop)
                    insts.insert(idx, nop)
                    idx += 1
                inst.sync_info = mybir.SyncInfo(
                    on_wait=keep, on_update=list(si.on_update))
                n_split += 1
            idx += 1
    return n_split


# ---------------------------------------------------------------------------
def _preprocess(edge_index, mask_idx):
    """Host-side graph partitioning: integer index work only."""
    src = np.asarray(edge_index[0], dtype=np.int64)
    dst = np.asarray(edge_index[1], dtype=np.int64)
    loop = np.arange(N, dtype=np.int64)
    src = np.concatenate([src, loop])
    dst = np.concatenate([dst, loop])

    core_of = dst // NSH
    block_of = (dst % NSH) // DB
    counts = np.zeros((NCORES, NB), dtype=np.int64)
    buckets = [[[] for _ in range(NB)] for _ in range(NCORES)]
    for e in range(len(src)):
        c, b = int(core_of[e]), int(block_of[e])
        buckets[c][b].append(e)
        counts[c][b] += 1
    TB = int(np.ceil(counts.max() / P))
    NT = NB * TB
    S = NT * P

    def to2d(a):
        return np.ascontiguousarray(a.reshape(NT, P).T)

    per_core = []
    for c in range(NCORES):
        esrc = np.zeros(S, dtype=np.int32)
        dloc = np.full(S, 1000.0, dtype=np.float32)
        for b in range(NB):
            base = b * TB * P
            for j, e in enumerate(buckets[c][b]):
                esrc[base + j] = src[e]
                dloc[base + j] = (dst[e] % NSH) % DB
        # src ids split by node-half for the two h2 allgather tables;
        # 30000 marks "other half" (skipped via bounds_check)
        low = (esrc % NSH) < 512
        rowh = (esrc // NSH) * 512 + (esrc % 512)
        esrcA = np.where(low, rowh, 30000).astype(np.int32)
        esrcB = np.where(~low, rowh, 30000).astype(np.int32)
        per_core.append(dict(esrc2d=to2d(esrc), esrcA2d=to2d(esrcA),
                             esrcB2d=to2d(esrcB), dloc2d=to2d(dloc)))

    mask = np.asarray(mask_idx, dtype=np.int64)
    mcore = mask // NSH
    positions = [np.where(mcore == c)[0] for c in range(NCORES)]
    MC = max(P, int(np.ceil(max(len(p) for p in positions) / P)) * P)
    MB = MC // DB
    # layer-2 aggregation only needs masked dst: re-bucket edges by mask slot
    in_edges = [[] for _ in range(N)]
    for e in range(len(src)):
        in_edges[int(dst[e])].append(int(src[e]))
    cnt2 = np.zeros((NCORES, MB), dtype=np.int64)
    for c in range(NCORES):
        pos = positions[c]
        for j in range(len(pos)):
            cnt2[c][j // DB] += len(in_edges[int(mask[pos[j]])])
    TB2 = int(np.ceil(cnt2.max() / P))
    NT2 = MB * TB2
    for c in range(NCORES):
        pos = positions[c]
        mloc = np.zeros(MC, dtype=np.int64)
        mnode = np.zeros(MC, dtype=np.int64)
        mloc[:len(pos)] = mask[pos] % NSH
        mnode[:len(pos)] = mask[pos]
        per_core[c]["mnode2d"] = np.ascontiguousarray(
            mnode.reshape(MC // P, P).T.astype(np.int32))
        per_core[c]["mblk2d"] = np.ascontiguousarray(
            mloc.reshape(MB, DB).T.astype(np.int32))
        esrc2 = np.zeros(NT2 * P, dtype=np.int32)
        dloc2 = np.full(NT2 * P, 1000.0, dtype=np.float32)
        for b2 in range(MB):
            base = b2 * TB2 * P
            k = 0
            for j in range(b2 * DB, min((b2 + 1) * DB, len(pos))):
                for s in in_edges[int(mask[pos[j]])]:
                    esrc2[base + k] = s
                    dloc2[base + k] = j % DB
                    k += 1
        per_core[c]["esrc2m"] = np.ascontiguousarray(
            esrc2.reshape(NT2, P).T)
        per_core[c]["dloc2m"] = np.ascontiguousarray(
            dloc2.reshape(NT2, P).T)
    return per_core, positions, TB, MC, TB2


# ---------------------------------------------------------------------------
def _build_program(TB, MC, TB2, b1_zero=False, dbg=False):
    NT = NB * TB
    MB = MC // DB
    NT2 = MB * TB2
    nc = bass.Bass("TRN2", target_bir_lowering=False, debug=False,
                   num_devices=NCORES)
    dp = lambda name, shape, dt: nc.declare_dram_parameter(
        name, list(shape), dt, isOutput=False)

    Xbf = dp("Xbf", [N, D], bf16)
    XTloc = dp("XTloc", [D, NSH], bf16)              # per-core X.T slice
    w1ain = dp("w1ain", [P, F * 16], bf16)           # host-folded att basis
    W1bf = dp("W1bf", [D, H1 * D], fp8 if FP8_S4 else bf16)
    W2bf = dp("W2bf", [H1 * D, D], fp8 if FP8_S5 else bf16)
    att2sb = dp("att2sb", [P, F * 2], bf16)
    fcwbf = dp("fcwbf", [2 * D, D], bf16)
    clswbf = dp("clswbf", [D, 2], bf16)
    b1in = dp("b1in", [P, 48], f32)
    b2bcin = dp("b2bcin", [P, D], f32)
    colsmin = dp("colsmin", [P, F], f32)
    fcbin = dp("fcbin", [P, F], f32)
    clsbin = dp("clsbin", [2, 1], f32)
    iotar = dp("iotar", [P, DB], f32)
    esrc2d_d = dp("esrc2d", [P, NT], i32)
    esrcA2d_d = dp("esrcA2d", [P, NT], i32)
    esrcB2d_d = dp("esrcB2d", [P, NT], i32)
    dloc2d_d = dp("dloc2d", [P, NT], f32)
    mblk2d_d = dp("mblk2d", [DB, MC // DB], i32)
    mnode2d_d = dp("mnode2d", [P, MC // P], i32)
    esrc2m_d = dp("esrc2m", [P, NT2], i32)
    dloc2m_d = dp("dloc2m", [P, NT2], f32)

    out_t = nc.declare_dram_parameter("out_t", [2, MC], f32, isOutput=True)
    dbg_outs = {}
    if dbg:
        for nm, shp, dt_ in [("dbg_a1", [N, 16], bf16),
                             ("dbg_x2t", [H1 * D, NSH], f32),
                             ("dbg_h2", [NSH, D], f32),
                             ("dbg_a2", [NSH, 2], bf16),
                             ("dbg_o2", [NSH, D], f32)]:
            dbg_outs[nm] = nc.declare_dram_parameter(nm, shp, dt_, isOutput=True)

    CORE_IDS = list(range(NCORES))

    with tile.TileContext(nc) as tc:
        with tc.tile_pool(name="const", bufs=1) as cpool, \
             tc.tile_pool(name="big", bufs=1) as bigpool, \
             tc.tile_pool(name="work", bufs=2) as wpool, \
             tc.tile_pool(name="edge", bufs=TB + 1) as epool, \
             tc.tile_pool(name="stream", bufs=3) as spool, \
             tc.tile_pool(name="dram", bufs=1, space="DRAM") as dpool:

            # ---- resident tables ----
            ident = cpool.tile([P, P], f32)
            make_identity(nc, ident[:])
            identbf = cpool.tile([P, P], bf16)
            nc.vector.tensor_copy(out=identbf[:], in_=ident[:])
            onesbf = cpool.tile([P, 1], bf16)
            nc.vector.memset(onesbf[:], 1.0)
            esrc_sb = cpool.tile([P, NT], i32)
            nc.sync.dma_start(out=esrc_sb[:], in_=esrc2d_d[:])
            esrcA_sb = cpool.tile([P, NT], i32)
            nc.sync.dma_start(out=esrcA_sb[:], in_=esrcA2d_d[:])
            esrcB_sb = cpool.tile([P, NT], i32)
            nc.sync.dma_start(out=esrcB_sb[:], in_=esrcB2d_d[:])
            dloc_sb = cpool.tile([P, NT], f32)
            nc.sync.dma_start(out=dloc_sb[:], in_=dloc2d_d[:])
            iotar_sb = cpool.tile([P, DB], f32)
            nc.sync.dma_start(out=iotar_sb[:], in_=iotar[:])
            w1a_sb = cpool.tile([P, F, 16], bf16)
            nc.sync.dma_start(out=w1a_sb[:],
                              in_=w1ain[:].rearrange("p (k j) -> p k j", j=16))
            att2_sb = cpool.tile([P, F, 2], bf16)
            nc.sync.dma_start(out=att2_sb[:],
                              in_=att2sb[:].rearrange("p (m j) -> p m j", j=2))
            b1_sb = cpool.tile([P, 48], f32)
            nc.sync.dma_start(out=b1_sb[:], in_=b1in[:])
            b2bc_sb = cpool.tile([P, D], f32)
            nc.sync.dma_start(out=b2bc_sb[:], in_=b2bcin[:])
            colsm_sb = cpool.tile([P, F], f32)
            nc.sync.dma_start(out=colsm_sb[:], in_=colsmin[:])
            fcb_sb = cpool.tile([P, F], f32)
            nc.sync.dma_start(out=fcb_sb[:], in_=fcbin[:])
            clsb_sb = cpool.tile([2, 1], f32)
            nc.sync.dma_start(out=clsb_sb[:], in_=clsbin[:])
            mblk_sb = cpool.tile([DB, MC // DB], i32)
            nc.sync.dma_start(out=mblk_sb[:], in_=mblk2d_d[:])
            esrc2m_sb = cpool.tile([P, NT2], i32)
            nc.sync.dma_start(out=esrc2m_sb[:], in_=esrc2m_d[:])
            dloc2m_sb = cpool.tile([P, NT2], f32)
            nc.sync.dma_start(out=dloc2m_sb[:], in_=dloc2m_d[:])
            mnode_sb = cpool.tile([P, MC // P], i32)
            nc.sync.dma_start(out=mnode_sb[:], in_=mnode2d_d[:])

            # ---- persistent big SBUF tensors ----
            h2t_sb = bigpool.tile([P, F, NSH], bf16)
            p1_sb = bigpool.tile([P, NT, H1], bf16)
            a01_all = bigpool.tile([P, NT, DB], bf16)
            a01t_all = bigpool.tile([DB, NT, P], bf16)

            # ---- internal DRAM ----
            a1loc = dpool.tile([NSH, 16], bf16)
            a1_dram = dpool.tile([N, 16], bf16, addr_space="Shared")
            x2t_dram = dpool.tile([H1 * D, NSH], fp8 if FP8_S5 else bf16)
            h2loc = dpool.tile([NSH, HW], bf16)
            a2loc = dpool.tile([NSH, 2], bf16)
            h2ag = dpool.tile([N, HW], bf16, addr_space="Shared")
            out2row = dpool.tile([MC, D], f32)

            # ============ S2: local a1 = X_loc @ w1a, AllGather =============
            with tc.tile_pool(name="ps2", bufs=2, space="PSUM") as ps2:
                a1r_sb = wpool.tile([P, NSH // P, 16], bf16, tag="a1r", bufs=1)
                for ch in range(NSH // 512):
                    a1t_ps = ps2.tile([16, 512], f32, tag="a1t", bufs=2)
                    for kt in range(F):
                        xt_t = spool.tile([P, 512], bf16, tag="xt")
                        nc.sync.dma_start(
                            out=xt_t[:],
                            in_=XTloc[kt * P:(kt + 1) * P,
                                      ch * 512:(ch + 1) * 512])
                        nc.tensor.matmul(
                            out=a1t_ps[:], lhsT=w1a_sb[:, kt, :],
                            rhs=xt_t[:],
                            start=(kt == 0), stop=(kt == F - 1))
                    a1t_sb = wpool.tile([16, 512], f32, tag="a1ts")
                    nc.vector.tensor_copy(out=a1t_sb[:], in_=a1t_ps[:])
                    for q in range(4):
                        tr_ps = ps2.tile([P, 16], f32, tag="tr16", bufs=2)
                        nc.tensor.transpose(out=tr_ps[:],
                                            in_=a1t_sb[:16, q * P:(q + 1) * P],
                                            identity=ident[:16, :16])
                        nc.vector.tensor_copy(out=a1r_sb[:, ch * 4 + q, :],
                                              in_=tr_ps[:])
                nc.sync.dma_start(
                    out=a1loc[:].rearrange("(a p) b -> p a b", p=P),
                    in_=a1r_sb[:])
            nc.gpsimd.collective_compute(
                "AllGather", mybir.AluOpType.bypass,
                replica_groups=[CORE_IDS], ins=[a1loc[:]], outs=[a1_dram[:]])
            if dbg:
                nc.sync.dma_start(out=dbg_outs["dbg_a1"][:], in_=a1_dram[:])

            # a1 rows of the core's own dst blocks (for dst-side scores)
            a1blk_sb = cpool.tile([DB, NB, 8], bf16)
            for b in range(NB):
                nc.sync.dma_start(out=a1blk_sb[:, b, :],
                                  in_=a1loc[b * DB:(b + 1) * DB, 8:16])

            # ============ S3+S4: layer 1, processed per 512-dst half ========
            for half in range(2):
              aggT_sb = bigpool.tile([P, F, H1, NSH // 2],
                                     fp8 if FP8_S4 else bf16, tag="aggT",
                                     bufs=2, name="aggT")
              with tc.tile_pool(name=f"ps3_{half}", bufs=1, space="PSUM") as ps3:
                for b in range(half * (NB // 2), (half + 1) * (NB // 2)):
                    aalls = []
                    dn_ps = ps3.tile([DB, H1], f32, tag="dn", bufs=1)
                    xgblk = epool.tile([P, TB, D], bf16, tag="xg", bufs=2)
                    g1blk = epool.tile([P, TB, 16], bf16, tag="g1", bufs=2)
                    adb_ps = ps3.tile([P, TB, H1], f32, tag="ad", bufs=2)
                    for tt in range(TB):
                        t = b * TB + tt
                        nc.gpsimd.indirect_dma_start(
                            out=xgblk[:, tt, :], out_offset=None, in_=Xbf[:],
                            in_offset=bass.IndirectOffsetOnAxis(
                                ap=esrc_sb[:, t:t + 1], axis=0))
                        nc.gpsimd.indirect_dma_start(
                            out=g1blk[:, tt, :], out_offset=None, in_=a1_dram[:],
                            in_offset=bass.IndirectOffsetOnAxis(
                                ap=esrc_sb[:, t:t + 1], axis=0))
                        # one-hot of local dst + its transpose (persisted)
                        nc.vector.tensor_scalar(out=a01_all[:, t, :],
                                                in0=iotar_sb[:],
                                                scalar1=dloc_sb[:, t:t + 1],
                                                scalar2=None,
                                                op0=mybir.AluOpType.is_equal)
                        trA_ps = ps3.tile([DB, P], bf16, tag="trA", bufs=1)
                        nc.tensor.transpose(out=trA_ps[:], in_=a01_all[:, t, :],
                                            identity=identbf[:])
                        nc.vector.tensor_copy(out=a01t_all[:, t, :],
                                              in_=trA_ps[:])
                        # dst-side score via one-hot broadcast matmul
                        nc.tensor.matmul(out=adb_ps[:, tt, :],
                                         lhsT=a01t_all[:, t, :],
                                         rhs=a1blk_sb[:, b, :],
                                         start=True, stop=True)
                    s_sb = epool.tile([P, TB, H1], f32, tag="s", bufs=2)
                    nc.vector.tensor_tensor(out=s_sb[:],
                                            in0=g1blk[:, :, :H1],
                                            in1=adb_ps[:],
                                            op=mybir.AluOpType.add)
                    lk_sb = epool.tile([P, TB, H1], f32, tag="lk", bufs=2)
                    nc.vector.scalar_tensor_tensor(
                        out=lk_sb[:], in0=s_sb[:], scalar=NEG_SLOPE,
                        in1=s_sb[:], op0=mybir.AluOpType.mult,
                        op1=mybir.AluOpType.max)
                    nc.scalar.activation(
                        out=p1_sb[:, b * TB:(b + 1) * TB, :], in_=lk_sb[:],
                        func=mybir.ActivationFunctionType.Exp)
                    for tt in range(TB):
                        t = b * TB + tt
                        nc.tensor.matmul(out=dn_ps[:], lhsT=a01_all[:, t, :],
                                         rhs=p1_sb[:, t, :],
                                         start=(tt == 0), stop=(tt == TB - 1))
                    recip_sb = wpool.tile([DB, H1], bf16, tag="recip")
                    with nc.allow_low_precision(reason="softmax denom in bf16"):
                        nc.vector.reciprocal(out=recip_sb[:], in_=dn_ps[:])
                    reb_ps = ps3.tile([P, TB, H1], f32, tag="re", bufs=1)
                    for tt in range(TB):
                        t = b * TB + tt
                        nc.tensor.matmul(out=reb_ps[:, tt, :],
                                         lhsT=a01t_all[:, t, :],
                                         rhs=recip_sb[:], start=True, stop=True)
                    alphab = epool.tile([P, TB, H1], f32, tag="alpha", bufs=2)
                    nc.vector.tensor_tensor(
                        out=alphab[:], in0=p1_sb[:, b * TB:(b + 1) * TB, :],
                        in1=reb_ps[:], op=mybir.AluOpType.mult)
                    for tt in range(TB):
                        t = b * TB + tt
                        a_all = epool.tile([P, H1 * DB], bf16, tag="aall")
                        for h in range(H1):
                            if h % 2 == 0:
                                nc.vector.tensor_scalar(
                                    out=a_all[:, h * DB:(h + 1) * DB],
                                    in0=a01_all[:, t, :],
                                    scalar1=alphab[:, tt, h:h + 1], scalar2=None,
                                    op0=mybir.AluOpType.mult)
                            else:
                                nc.scalar.activation(
                                    out=a_all[:, h * DB:(h + 1) * DB],
                                    in_=a01_all[:, t, :],
                                    func=mybir.ActivationFunctionType.Copy,
                                    scale=alphab[:, tt, h:h + 1])
                        aalls.append(a_all)
                    for fs in range(2):
                        for f in range(fs * 3, fs * 3 + 3):
                            ag_ps = ps3.tile([P, H1 * DB], f32, tag=f"agg{f % 3}",
                                             bufs=1)
                            for tt in range(TB):
                                nc.tensor.matmul(
                                    out=ag_ps[:],
                                    lhsT=xgblk[:, tt, f * P:(f + 1) * P],
                                    rhs=aalls[tt][:],
                                    start=(tt == 0), stop=(tt == TB - 1))
                            agdst = aggT_sb[:, f, :,
                                            (b - half * (NB // 2)) * DB:
                                            (b + 1 - half * (NB // 2)) * DB]
                            agsrc = ag_ps[:].rearrange("p (h d) -> p h d", h=H1)
                            if f % 2 == 1:
                                nc.scalar.activation(
                                    out=agdst, in_=agsrc,
                                    func=mybir.ActivationFunctionType.Copy)
                            else:
                                nc.vector.tensor_copy(out=agdst, in_=agsrc)

              # ===== S4 (per half): out1 = aggT @ W1, +b1, elu -> x2t_dram ==
              inv_s4 = 1.0 / WSCALE if FP8_S4 else 1.0
              w1dt = fp8 if FP8_S4 else bf16
              with tc.tile_pool(name=f"ps4_{half}", bufs=1, space="PSUM") as ps4:
                for h in range(H1):
                    for m in range(F):
                        w1hm = spool.tile([P, F, P], w1dt, tag="w1hm", bufs=2)
                        nc.sync.dma_start(
                            out=w1hm[:],
                            in_=W1bf[:].rearrange("(k p) n -> p k n", p=P)
                            [:, :, h * D + m * P:h * D + (m + 1) * P])
                        o1_ps = ps4.tile([P, 512], f32, tag="o1", bufs=2)
                        if FP8_S4:
                            for k2 in range(F // 2):
                                nc.tensor.matmul(
                                    out=o1_ps[:],
                                    lhsT=w1hm[:, 2 * k2:2 * k2 + 2, :],
                                    rhs=aggT_sb[:, 2 * k2:2 * k2 + 2, h, :],
                                    start=(k2 == 0), stop=(k2 == F // 2 - 1),
                                    perf_mode=mybir.MatmulPerfMode.DoubleRow)
                        else:
                            for kt in range(F):
                                nc.tensor.matmul(
                                    out=o1_ps[:], lhsT=w1hm[:, kt, :],
                                    rhs=aggT_sb[:, kt, h, :],
                                    start=(kt == 0), stop=(kt == F - 1))
                        j = h * F + m
                        ebuf = wpool.tile([P, 512], bf16, tag="ebuf")
                        nc.scalar.activation(
                            out=ebuf[:], in_=o1_ps[:],
                            func=mybir.ActivationFunctionType.Exp,
                            bias=b1_sb[:, j:j + 1], scale=inv_s4)
                        t1a = wpool.tile([P, 512], bf16, tag="t1a")
                        nc.scalar.activation(
                            out=t1a[:], in_=o1_ps[:],
                            func=mybir.ActivationFunctionType.Relu,
                            bias=b1_sb[:, j:j + 1], scale=inv_s4)
                        # x2' = x2 + 1 = relu(xb) + min(exp(xb), 1); the +1
                        # is compensated by subtracting colsum(W2) from h2
                        x2t = wpool.tile([P, 512], fp8 if FP8_S5 else bf16,
                                         tag="x2t")
                        nc.vector.scalar_tensor_tensor(
                            out=x2t[:], in0=ebuf[:], scalar=1.0, in1=t1a[:],
                            op0=mybir.AluOpType.min, op1=mybir.AluOpType.add)
                        nc.sync.dma_start(
                            out=x2t_dram[h * D + m * P:h * D + (m + 1) * P,
                                         half * 512:(half + 1) * 512],
                            in_=x2t[:])
            if dbg:
                for kt in range(48):
                    xx = wpool.tile([P, NSH], fp8 if FP8_S5 else bf16, tag="dbgx")
                    nc.sync.dma_start(out=xx[:], in_=x2t_dram[kt * P:(kt + 1) * P, :])
                    xxf = wpool.tile([P, NSH], f32, tag="dbgxf")
                    nc.vector.tensor_copy(out=xxf[:], in_=xx[:])
                    nc.sync.dma_start(out=dbg_outs["dbg_x2t"][kt * P:(kt + 1) * P, :],
                                      in_=xxf[:])

            # ============ S5: h2 = x2 @ W2, transposes, a2 ==================
            inv_s5 = 1.0 / WSCALE if FP8_S5 else 1.0
            with tc.tile_pool(name="ps5", bufs=1, space="PSUM") as ps5:
                for ch in range(2):
                    h2_ps = [ps5.tile([P, 512], f32, tag=f"h2_{m}", bufs=1,
                                      name=f"h2ps{m}")
                             for m in range(F)]
                    if FP8_S5:
                        for k2 in range(24):
                            x2tt = spool.tile([P, 2, 512], fp8, tag="x2rd")
                            nc.sync.dma_start(
                                out=x2tt[:],
                                in_=x2t_dram[:].rearrange("(k p) n -> p k n", p=P)
                                [:, 2 * k2:2 * k2 + 2, ch * 512:(ch + 1) * 512])
                            w2kt = spool.tile([P, 2, D], fp8, tag="w2kt")
                            nc.sync.dma_start(
                                out=w2kt[:],
                                in_=W2bf[:].rearrange("(k p) n -> p k n", p=P)
                                [:, 2 * k2:2 * k2 + 2, :])
                            for m in range(F):
                                nc.tensor.matmul(
                                    out=h2_ps[m][:],
                                    lhsT=w2kt[:, :, m * P:(m + 1) * P],
                                    rhs=x2tt[:], start=(k2 == 0),
                                    stop=(k2 == 23),
                                    perf_mode=mybir.MatmulPerfMode.DoubleRow)
                    else:
                        for kt in range(48):
                            x2tt = spool.tile([P, 512], bf16, tag="x2rd")
                            nc.sync.dma_start(
                                out=x2tt[:],
                                in_=x2t_dram[kt * P:(kt + 1) * P,
                                             ch * 512:(ch + 1) * 512])
                            w2kt = spool.tile([P, D], bf16, tag="w2kt")
                            nc.sync.dma_start(
                                out=w2kt[:],
                                in_=W2bf[:].rearrange("(k p) n -> p k n", p=P)
                                [:, kt, :])
                            for m in range(F):
                                nc.tensor.matmul(out=h2_ps[m][:],
                                                 lhsT=w2kt[:, m * P:(m + 1) * P],
                                                 rhs=x2tt[:], start=(kt == 0),
                                                 stop=(kt == 47))
                    for m in range(F):
                        nc.vector.tensor_scalar(
                            out=h2t_sb[:, m, ch * 512:(ch + 1) * 512],
                            in0=h2_ps[m][:], scalar1=inv_s5,
                            scalar2=colsm_sb[:, m:m + 1],
                            op0=mybir.AluOpType.mult,
                            op1=mybir.AluOpType.subtract)
                    for m in range(F):
                        for q in range(4):
                            c0 = ch * 512 + q * P
                            trh_ps = ps5.tile([P, P], bf16, tag="trh", bufs=1)
                            nc.tensor.transpose(out=trh_ps[:],
                                                in_=h2t_sb[:, m, c0:c0 + P],
                                                identity=identbf[:])
                            h2r = wpool.tile([P, P], bf16, tag="h2r")
                            nc.vector.tensor_copy(out=h2r[:], in_=trh_ps[:])
                            nc.sync.dma_start(
                                out=h2loc[c0:c0 + P, m * P:(m + 1) * P], in_=h2r[:])
                    for dt_ in range(4):
                        c0 = ch * 512 + dt_ * P
                        a2_ps = ps5.tile([P, 2], f32, tag="a2", bufs=1)
                        for m in range(F):
                            nc.tensor.matmul(out=a2_ps[:],
                                             lhsT=h2t_sb[:, m, c0:c0 + P],
                                             rhs=att2_sb[:, m, :],
                                             start=(m == 0), stop=(m == F - 1))
                        a2b = wpool.tile([P, 2], bf16, tag="a2b")
                        nc.vector.tensor_copy(out=a2b[:], in_=a2_ps[:])
                        nc.sync.dma_start(out=a2loc[c0:c0 + P, :], in_=a2b[:])
                        nc.sync.dma_start(out=h2loc[c0:c0 + P, D:D + 2], in_=a2b[:])
                        nc.sync.dma_start(out=h2loc[c0:c0 + P, D + 2:D + 3],
                                          in_=onesbf[:])
            if dbg:
                nc.sync.dma_start(out=dbg_outs["dbg_a2"][:], in_=a2loc[:])
                for q in range(8):
                    h2b = wpool.tile([P, D], bf16, tag="dbgx")
                    nc.sync.dma_start(out=h2b[:], in_=h2loc[q * P:(q + 1) * P, :D])
                    h2f = wpool.tile([P, D], f32, tag="dbgxf")
                    nc.vector.tensor_copy(out=h2f[:], in_=h2b[:])
                    nc.sync.dma_start(out=dbg_outs["dbg_h2"][q * P:(q + 1) * P, :],
                                      in_=h2f[:])

            # ============ S6: AllGather h2 rows (feat | a_s | a_d | 1) ======
            nc.gpsimd.collective_compute(
                "AllGather", mybir.AluOpType.bypass,
                replica_groups=[CORE_IDS], ins=[h2loc[:]], outs=[h2ag[:]])

            # a2 dst rows of the masked slots (gathered by local node id)
            a2blk_sb = cpool.tile([DB, MC // DB, 2], bf16)
            for b2 in range(MC // DB):
                nc.gpsimd.indirect_dma_start(
                    out=a2blk_sb[:, b2, :], out_offset=None, in_=a2loc[:],
                    in_offset=bass.IndirectOffsetOnAxis(
                        ap=mblk_sb[:, b2:b2 + 1], axis=0))

            # ===== S8 prep: x_emb gathers + transposes (overlap with S7) ====
            with tc.tile_pool(name="ps8a", bufs=1, space="PSUM") as ps8a:
                xem_bf = bigpool.tile([P, F, MC], bf16)
                for q in range(MC // P):
                    xe_r = wpool.tile([P, D], bf16, tag="xer")
                    nc.gpsimd.indirect_dma_start(
                        out=xe_r[:], out_offset=None, in_=Xbf[:],
                        in_offset=bass.IndirectOffsetOnAxis(
                            ap=mnode_sb[:, q:q + 1], axis=0))
                    for f in range(F):
                        tre_ps = ps8a.tile([P, P], bf16, tag="tre", bufs=2)
                        nc.tensor.transpose(out=tre_ps[:],
                                            in_=xe_r[:, f * P:(f + 1) * P],
                                            identity=identbf[:])
                        nc.vector.tensor_copy(out=xem_bf[:, f, q * P:(q + 1) * P],
                                              in_=tre_ps[:])

            # ============ S7: layer-2 edge phase, masked dst only =======
            with tc.tile_pool(name="ps7", bufs=1, space="PSUM") as ps7:
                for b2 in range(MB):
                    outA_ps = ps7.tile([DB, 512], f32, tag="outA", bufs=1)
                    outB_ps = ps7.tile([DB, 256], f32, tag="outB", bufs=1)
                    outD_ps = ps7.tile([DB, 1], f32, tag="outD", bufs=1)
                    hgblk = epool.tile([P, TB2, HW], bf16, tag="hg", bufs=2)
                    ad2b_ps = ps7.tile([P, TB2, 1], f32, tag="ad2", bufs=2)
                    a01m = epool.tile([P, TB2, DB], bf16, tag="a01m", bufs=2)
                    for tt in range(TB2):
                        t = b2 * TB2 + tt
                        nc.gpsimd.indirect_dma_start(
                            out=hgblk[:, tt, :], out_offset=None, in_=h2ag[:],
                            in_offset=bass.IndirectOffsetOnAxis(
                                ap=esrc2m_sb[:, t:t + 1], axis=0))
                        nc.vector.tensor_scalar(out=a01m[:, tt, :],
                                                in0=iotar_sb[:],
                                                scalar1=dloc2m_sb[:, t:t + 1],
                                                scalar2=None,
                                                op0=mybir.AluOpType.is_equal)
                        trm_ps = ps7.tile([DB, P], bf16, tag="trm", bufs=2)
                        nc.tensor.transpose(out=trm_ps[:], in_=a01m[:, tt, :],
                                            identity=identbf[:])
                        a01tm = epool.tile([DB, P], bf16, tag="a01tm")
                        nc.vector.tensor_copy(out=a01tm[:], in_=trm_ps[:])
                        nc.tensor.matmul(out=ad2b_ps[:, tt, :],
                                         lhsT=a01tm[:],
                                         rhs=a2blk_sb[:, b2, 1:2],
                                         start=True, stop=True)
                    s_sb = epool.tile([P, TB2, 1], f32, tag="s", bufs=2)
                    nc.vector.tensor_tensor(out=s_sb[:],
                                            in0=hgblk[:, :, D:D + 1],
                                            in1=ad2b_ps[:],
                                            op=mybir.AluOpType.add)
                    lk_sb = epool.tile([P, TB2, 1], f32, tag="lk", bufs=2)
                    nc.vector.scalar_tensor_tensor(
                        out=lk_sb[:], in0=s_sb[:], scalar=NEG_SLOPE,
                        in1=s_sb[:], op0=mybir.AluOpType.mult,
                        op1=mybir.AluOpType.max)
                    p2_sb = epool.tile([P, TB2, 1], f32, tag="p2", bufs=2)
                    nc.scalar.activation(out=p2_sb[:], in_=lk_sb[:],
                                         func=mybir.ActivationFunctionType.Exp)
                    for tt in range(TB2):
                        a_all = epool.tile([P, DB], bf16, tag="aall2")
                        if tt % 2 == 0:
                            nc.vector.tensor_scalar(
                                out=a_all[:], in0=a01m[:, tt, :],
                                scalar1=p2_sb[:, tt, 0:1], scalar2=None,
                                op0=mybir.AluOpType.mult)
                        else:
                            nc.scalar.activation(
                                out=a_all[:], in_=a01m[:, tt, :],
                                func=mybir.ActivationFunctionType.Copy,
                                scale=p2_sb[:, tt, 0:1])
                        nc.tensor.matmul(out=outA_ps[:], lhsT=a_all[:],
                                         rhs=hgblk[:, tt, 0:512],
                                         start=(tt == 0), stop=(tt == TB2 - 1))
                        nc.tensor.matmul(out=outB_ps[:], lhsT=a_all[:],
                                         rhs=hgblk[:, tt, 512:D],
                                         start=(tt == 0), stop=(tt == TB2 - 1))
                        nc.tensor.matmul(out=outD_ps[:], lhsT=a_all[:],
                                         rhs=hgblk[:, tt, D + 2:D + 3],
                                         start=(tt == 0), stop=(tt == TB2 - 1))
                    recd_sb = wpool.tile([DB, 1], f32, tag="recd")
                    nc.vector.reciprocal(out=recd_sb[:], in_=outD_ps[:])
                    o2_sb = wpool.tile([DB, D], f32, tag="o2sb")
                    nc.vector.scalar_tensor_tensor(
                        out=o2_sb[:, 0:512], in0=outA_ps[:],
                        scalar=recd_sb[:, 0:1], in1=b2bc_sb[:DB, 0:512],
                        op0=mybir.AluOpType.mult, op1=mybir.AluOpType.add)
                    nc.vector.scalar_tensor_tensor(
                        out=o2_sb[:, 512:D], in0=outB_ps[:],
                        scalar=recd_sb[:, 0:1], in1=b2bc_sb[:DB, 512:D],
                        op0=mybir.AluOpType.mult, op1=mybir.AluOpType.add)
                    nc.sync.dma_start(out=out2row[b2 * DB:(b2 + 1) * DB, :],
                                      in_=o2_sb[:])

            # ============ S8: masked fc + classifier ========================
            with tc.tile_pool(name="ps8", bufs=1, space="PSUM") as ps8:
                xgm_bf = bigpool.tile([P, F, MC], bf16)
                for q in range(MC // P):
                    xg_r = wpool.tile([P, D], f32, tag="xgr")
                    nc.sync.dma_start(out=xg_r[:],
                                      in_=out2row[q * P:(q + 1) * P, :])
                    for f in range(F):
                        trg_ps = ps8.tile([P, P], f32, tag="tro", bufs=2)
                        nc.tensor.transpose(out=trg_ps[:],
                                            in_=xg_r[:, f * P:(f + 1) * P],
                                            identity=ident[:])
                        nc.vector.tensor_copy(out=xgm_bf[:, f, q * P:(q + 1) * P],
                                              in_=trg_ps[:])
                clsw_sb = cpool.tile([P, F, 2], bf16)
                nc.sync.dma_start(out=clsw_sb[:],
                                  in_=clswbf[:].rearrange("(m p) n -> p m n", p=P))
                fcT_bf = bigpool.tile([P, F, MC], bf16)
                for m in range(F):
                    fcwm = spool.tile([P, 12, P], bf16, tag="fcwm", bufs=2)
                    nc.sync.dma_start(
                        out=fcwm[:],
                        in_=fcwbf[:].rearrange("(k p) n -> p k n", p=P)
                        [:, :, m * P:(m + 1) * P])
                    fc_ps = ps8.tile([P, MC], f32, tag="fc", bufs=2)
                    for kt in range(12):
                        rhs = xgm_bf[:, kt, :] if kt < F else xem_bf[:, kt - F, :]
                        nc.tensor.matmul(out=fc_ps[:], lhsT=fcwm[:, kt, :],
                                         rhs=rhs, start=(kt == 0), stop=(kt == 11))
                    nc.vector.tensor_scalar(out=fcT_bf[:, m, :], in0=fc_ps[:],
                                            scalar1=fcb_sb[:, m:m + 1],
                                            scalar2=None, op0=mybir.AluOpType.add)
                cls_ps = ps8.tile([2, MC], f32, tag="cls", bufs=1)
                for m in range(F):
                    nc.tensor.matmul(out=cls_ps[:], lhsT=clsw_sb[:, m, :],
                                     rhs=fcT_bf[:, m, :],
                                     start=(m == 0), stop=(m == F - 1))
                outf = wpool.tile([2, MC], f32, tag="outf")
                nc.vector.tensor_scalar(out=outf[:], in0=cls_ps[:],
                                        scalar1=clsb_sb[:, 0:1], scalar2=None,
                                        op0=mybir.AluOpType.add)
                nc.sync.dma_start(out=out_t[:], in_=outf[:])

    _split_excess_waits(nc)
    return nc


# ---------------------------------------------------------------------------
def kernel(cls_embeddings, edge_index, mask_idx, W1, att_src1, att_dst1, b1,
           W2, att_src2, att_dst2, b2, fc_w, fc_b, cls_w, cls_b, _dbg=False):
    X = np.asarray(cls_embeddings, dtype=np.float32)
    per_core, positions, TB, MC, TB2 = _preprocess(np.asarray(edge_index),
                                                   np.asarray(mask_idx))

    # host-folded attention basis: w1a[d, j] = sum_c W1[d, hc] att_j[h, c]
    W1f = np.asarray(W1, np.float32).reshape(D, H1, D)
    w1a = np.concatenate(
        [np.einsum("dhc,hc->dh", W1f, np.asarray(att_src1, np.float32)),
         np.einsum("dhc,hc->dh", W1f, np.asarray(att_dst1, np.float32))],
        axis=1)                                    # [768, 16]
    w1ain = np.ascontiguousarray(
        w1a.reshape(F, P, 16).transpose(1, 0, 2).reshape(P, F * 16))

    att2T = np.stack([np.asarray(att_src2, np.float32)[0],
                      np.asarray(att_dst2, np.float32)[0]], axis=1)
    att2_sb = np.ascontiguousarray(
        att2T.reshape(F, P, 2).transpose(1, 0, 2).reshape(P, F * 2))

    XT = np.ascontiguousarray(X.T).astype(BF)      # [768, 8192]

    W1h = np.asarray(W1, np.float32)
    W2h = np.asarray(W2, np.float32)
    shared = {
        "Xbf": X.astype(BF),
        "w1ain": w1ain.astype(BF),
        "W1bf": (W1h * WSCALE).astype(F8) if FP8_S4 else W1h.astype(BF),
        "W2bf": (W2h * WSCALE).astype(F8) if FP8_S5 else W2h.astype(BF),
        "att2sb": att2_sb.astype(BF),
        "fcwbf": np.asarray(fc_w, np.float32).astype(BF),
        "clswbf": np.asarray(cls_w, np.float32).astype(BF),
        "b1in": np.ascontiguousarray(np.asarray(b1, np.float32).reshape(48, P).T),
        "b2bcin": np.tile(np.asarray(b2, np.float32).reshape(1, D), (P, 1)),
        "colsmin": np.ascontiguousarray(
            W2h.sum(axis=0, dtype=np.float64).astype(np.float32)
            .reshape(F, P).T),
        "fcbin": np.ascontiguousarray(np.asarray(fc_b, np.float32).reshape(F, P).T),
        "clsbin": np.asarray(cls_b, np.float32).reshape(2, 1),
        "iotar": np.tile(np.arange(DB, dtype=np.float32), (P, 1)),
    }

    b1_zero = bool(np.all(np.asarray(b1) == 0.0))
    nc = _build_program(TB, MC, TB2, b1_zero=b1_zero, dbg=_dbg)
    in_maps = []
    for c in range(NCORES):
        m = dict(shared)
        m.update(per_core[c])
        m["XTloc"] = np.ascontiguousarray(XT[:, c * NSH:(c + 1) * NSH])
        in_maps.append(m)

    global LAST
    kres = run_bass_kernel_spmd(nc, in_maps, list(range(NCORES)),
                                trace=TRACE, tmpdir=TRACE_DIR)
    LAST = kres
    res = kres.results

    out = np.zeros((M, 2), dtype=np.float32)
    for c in range(NCORES):
        pos = positions[c]
        ot = res[c]["out_t"]
        for j, p_ in enumerate(pos):
            out[p_] = ot[:, j]
    if _dbg:
        return out, res, positions
    return out



# revision 12
# speedup vs baseline: 1.9090x; 1.9090x over previous
"""Two-layer GAT (PyG-style GATConv) on 8 Trainium2 NeuronCores.

v3 layout (active-set + SBUF-resident intermediates):
- Only "active" nodes (srcs of edges into masked dsts, plus masked dsts)
  need layer-1 output / layer-2 features: ~45% of all nodes. Layer-1
  aggregation, S4 (agg@W1), S5 (x2@W2) and the h2 exchange are restricted
  to the per-core active set (padded to NACT=512 vs 1024 owned nodes).
- Single S3 pass over all dst blocks -> aggT (fp8) fully SBUF-resident;
  S4 streams W1 once, keeps x2t in SBUF (no DRAM round-trip); S5 streams
  W2 once against the SBUF-resident x2t.
- Indirect gathers batched per block (one SWDGE call per block instead of
  per 128-edge tile) and X-row gathers for all blocks issued up-front so
  they overlap S2 + the a1 AllGather.
- h2 AllGather carries only active rows ([NACT, 772] per core). S8 prep
  (x_emb gathers, fc weight loads, fc partial accumulation over the
  x_emb half) overlaps the collective; gpsimd-queue order keeps
  independent gathers ahead of the collective trigger.
- Masked fc/classifier on the owning core; host reassembles the output.
"""
import numpy as np
from concourse import bass, mybir
import concourse.tile as tile
from concourse.bass_utils import run_bass_kernel_spmd
from concourse.vector_clock import ScopedClock, VectorClock
from concourse.masks import make_identity

N, E, M = 8192, 32768, 1024
D = 768
F = 6               # 768 / 128
H1 = 8
NCORES = 8
NSH = N // NCORES   # 1024 nodes owned per core
DB = 64
P = 128
HW = D + 4          # h2 row: 768 h2 | a_s2 | a_d2 | 1.0 | pad

f32 = mybir.dt.float32
bf16 = mybir.dt.bfloat16
fp8 = mybir.dt.float8e4
i32 = mybir.dt.int32
BF = mybir.dt.np(bf16)
F8 = mybir.dt.np(fp8)
WSCALE = 32.0       # fp8 W1 pre-scale (undone via activation scale)

NEG_SLOPE = 0.2

# test-harness knobs (harness calls kernel() with defaults: no tracing)
TRACE = False
TRACE_DIR = None
LAST = None


# ---------------------------------------------------------------------------
# The walrus build in this container rejects a Drain instruction with more
# than one semaphore wait ("Too many sync wait commands"); the default
# TileContext kernel-tail drain has many. Emit one single-wait drain per
# logical processor instead.
def _split_drain_and_barrier(self, tick_clock, wait_clock):
    gc = tick_clock.global_clock
    nprocs = 27
    for i in range(nprocs):
        mask = [0] * nprocs
        mask[i] = 1 << 30
        part = gc.elementwise_min(VectorClock(mask))
        d = self.nc.sync.drain()
        wait_clock.add_sem_waits(d.ins, ScopedClock({None: part}))
    self.nc.all_engine_barrier()
    popped = self.nc._tile_sem_poison_stack.pop()
    assert popped is self._sem_poison
    self.nc.clear_and_free_semaphores(list(self.sems.allocated().values()))
    self.nc.all_engine_barrier()


tile.TileContext._drain_and_barrier = _split_drain_and_barrier

MAX_WAITS = 1  # this walrus build rejects multi-sem-wait instructions


def _split_excess_waits(nc):
    """Move excess semaphore waits onto preceding same-engine NoOps."""
    n_split = 0
    for bb in nc.m.functions[0].blocks:
        insts = bb.instructions
        idx = 0
        while idx < len(insts):
            inst = insts[idx]
            si = inst.sync_info
            if si is not None and len(si.on_wait) > MAX_WAITS:
                waits = list(si.on_wait)
                keep = waits[-MAX_WAITS:]
                extra = waits[:-MAX_WAITS]
                for gi in range(0, len(extra), MAX_WAITS):
                    nop = mybir.InstNoOp(
                        name=f"WSPLIT-{nc.next_id()}",
                        sync_info=mybir.SyncInfo(
                            on_wait=extra[gi:gi + MAX_WAITS], on_update=[]),
                        bass_nofuse=True,
                        engine=inst.engine,
                        ins=[], outs=[],
                    )
                    nc.register_instruction(nop)
                    insts.insert(idx, nop)
                    idx += 1
                inst.sync_info = mybir.SyncInfo(
                    on_wait=keep, on_update=list(si.on_update))
                n_split += 1
            idx += 1
    return n_split


# ---------------------------------------------------------------------------
def _preprocess(edge_index, mask_idx):
    """Host-side graph partitioning: integer index work only."""
    src = np.asarray(edge_index[0], dtype=np.int64)
    dst = np.asarray(edge_index[1], dtype=np.int64)
    loop = np.arange(N, dtype=np.int64)
    src = np.concatenate([src, loop])
    dst = np.concatenate([dst, loop])
    mask = np.asarray(mask_idx, dtype=np.int64)

    # active set: masked nodes + srcs of edges into masked nodes
    mset = np.zeros(N, dtype=bool)
    mset[mask] = True
    need = np.zeros(N, dtype=bool)
    need[src[mset[dst]]] = True
    need[mask] = True

    aslot = np.full(N, -1, dtype=np.int64)
    act_lists = []
    for c in range(NCORES):
        nodes = np.nonzero(need[c * NSH:(c + 1) * NSH])[0] + c * NSH
        act_lists.append(nodes)
        aslot[nodes] = np.arange(len(nodes))
    NACT = int(np.ceil(max(len(a) for a in act_lists) / 512)) * 512
    NB = NACT // DB
    gid = (np.arange(N) // NSH) * NACT + aslot   # row in h2ag (valid if need)

    # ---- layer-1 edges: those into active dsts, bucketed (core, block) ----
    keep = need[dst]
    e_src, e_dst = src[keep], dst[keep]
    e_core = e_dst // NSH
    e_slot = aslot[e_dst]
    bucket = e_core * NB + e_slot // DB
    counts = np.bincount(bucket, minlength=NCORES * NB)
    TB = int(np.ceil(counts.max() / P))
    NT1 = NB * TB
    order = np.argsort(bucket, kind='stable')
    starts = np.zeros(NCORES * NB + 1, dtype=np.int64)
    np.cumsum(counts, out=starts[1:])
    pos = np.arange(len(order)) - starts[bucket[order]]
    flat = np.zeros((NCORES, NB * TB * P), dtype=np.int64)      # src node ids
    dflat = np.full((NCORES, NB * TB * P), 1000.0, dtype=np.float32)
    bo = bucket[order]
    addr = (bo % NB) * TB * P + pos
    flat[e_core[order], addr] = e_src[order]
    dflat[e_core[order], addr] = (e_slot[order] % DB).astype(np.float32)

    def to2d(a, nt):
        return np.ascontiguousarray(a.reshape(nt, P).T)

    per_core = []
    for c in range(NCORES):
        per_core.append(dict(
            esrc2d=to2d(flat[c].astype(np.int32), NT1),
            dloc2d=to2d(dflat[c], NT1),
        ))

    # ---- masked nodes per owning core ----
    positions = [np.nonzero(mask // NSH == c)[0] for c in range(NCORES)]
    MC = max(P, int(np.ceil(max(len(p) for p in positions) / P)) * P)
    MB = MC // DB

    # layer-2 edges grouped per masked occurrence (src is always active)
    l2 = mset[dst]
    l2_src, l2_dst = src[l2], dst[l2]
    d_order = np.argsort(l2_dst, kind='stable')
    sd = l2_dst[d_order]
    cnt2 = np.zeros((NCORES, MB), dtype=np.int64)
    for c in range(NCORES):
        pos_c = positions[c]
        nodes = mask[pos_c]
        lo = np.searchsorted(sd, nodes, side='left')
        hi = np.searchsorted(sd, nodes, side='right')
        deg = hi - lo
        nb2 = np.minimum(np.arange(len(pos_c)) // DB, MB - 1)
        np.add.at(cnt2[c], nb2, deg)
    TB2 = int(np.ceil(cnt2.max() / P))
    NT2 = MB * TB2
    for c in range(NCORES):
        pos_c = positions[c]
        nodes = mask[pos_c]
        mnode = np.zeros(MC, dtype=np.int64)
        mnode[:len(pos_c)] = nodes
        mloc = np.zeros(MC, dtype=np.int64)
        mloc[:len(pos_c)] = aslot[nodes]
        per_core[c]["mnode2d"] = np.ascontiguousarray(
            mnode.reshape(MC // P, P).T.astype(np.int32))
        per_core[c]["mblk2d"] = np.ascontiguousarray(
            mloc.reshape(MB, DB).T.astype(np.int32))
        esrc2 = np.zeros(NT2 * P, dtype=np.int32)
        dloc2 = np.full(NT2 * P, 1000.0, dtype=np.float32)
        lo = np.searchsorted(sd, nodes, side='left')
        hi = np.searchsorted(sd, nodes, side='right')
        for b2 in range(MB):
            base = b2 * TB2 * P
            k = 0
            for j in range(b2 * DB, min((b2 + 1) * DB, len(pos_c))):
                for e in d_order[lo[j]:hi[j]]:
                    esrc2[base + k] = gid[l2_src[e]]
                    dloc2[base + k] = j % DB
                    k += 1
        per_core[c]["esrc2m"] = to2d(esrc2, NT2)
        per_core[c]["dloc2m"] = to2d(dloc2, NT2)
    return per_core, positions, act_lists, NACT, TB, MC, TB2


# ---------------------------------------------------------------------------
def _build_program(NACT, TB, MC, TB2, dbg=False):
    NB = NACT // DB
    NC_CH = NACT // 512          # 512-wide chunks of the active set
    MB = MC // DB
    NT1 = NB * TB
    NT2 = MB * TB2
    nc = bass.Bass("TRN2", target_bir_lowering=False, debug=False,
                   num_devices=NCORES)
    dp = lambda name, shape, dt: nc.declare_dram_parameter(
        name, list(shape), dt, isOutput=False)

    Xbf = dp("Xbf", [N, D], bf16)
    XTloc = dp("XTloc", [P, F, NSH], bf16)           # per-core X.T (own nodes)
    XTact = dp("XTact", [P, F, NACT], bf16)          # per-core X.T (active)
    w1ain = dp("w1ain", [P, F * 16], bf16)           # host-folded att basis
    W1r = dp("W1r", [P, 48 * F * P], fp8)            # [p, j, kt, q] fp8*WSCALE
    W2r = dp("W2r", [P, 48 * D], bf16)               # [p, kt, n]
    att2sb = dp("att2sb", [P, F * 2], bf16)
    fcwr = dp("fcwr", [P, F * 12 * P], bf16)         # [p, m, kt, q]
    clswbf = dp("clswbf", [D, 2], bf16)
    b1in = dp("b1in", [P, 48], f32)
    b2bcin = dp("b2bcin", [P, D], f32)
    colsmin = dp("colsmin", [P, F], f32)
    fcbin = dp("fcbin", [P, F], f32)
    clsbin = dp("clsbin", [2, 1], f32)
    iotar = dp("iotar", [P, DB], f32)
    esrc2d_d = dp("esrc2d", [P, NT1], i32)
    dloc2d_d = dp("dloc2d", [P, NT1], f32)
    mblk2d_d = dp("mblk2d", [DB, MB], i32)
    mnode2d_d = dp("mnode2d", [P, MC // P], i32)
    esrc2m_d = dp("esrc2m", [P, NT2], i32)
    dloc2m_d = dp("dloc2m", [P, NT2], f32)

    out_t = nc.declare_dram_parameter("out_t", [2, MC], f32, isOutput=True)
    dbg_outs = {}
    if dbg:
        for nm, shp, dt_ in [("dbg_a1", [N, 16], bf16),
                             ("dbg_agg", [F * P, H1 * NACT], f32),
                             ("dbg_x2t", [48 * P, NACT], f32),
                             ("dbg_h2", [NACT, HW], f32),
                             ("dbg_o2", [MC, D], f32)]:
            dbg_outs[nm] = nc.declare_dram_parameter(nm, shp, dt_, isOutput=True)

    CORE_IDS = list(range(NCORES))

    with tile.TileContext(nc) as tc:
        with tc.tile_pool(name="const", bufs=1) as cpool, \
             tc.tile_pool(name="big", bufs=1) as bigpool, \
             tc.tile_pool(name="work", bufs=2) as wpool, \
             tc.tile_pool(name="edge", bufs=2) as epool, \
             tc.tile_pool(name="stream", bufs=3) as spool, \
             tc.tile_pool(name="dram", bufs=1, space="DRAM") as dpool:

            # ---- big streams first on the DMA queue ----
            xt_sb = cpool.tile([P, F, NSH], bf16)
            nc.sync.dma_start(out=xt_sb[:], in_=XTloc[:])
            xta_sb = cpool.tile([P, F, NACT], bf16)
            nc.sync.dma_start(out=xta_sb[:], in_=XTact[:])
            w1a_sb = cpool.tile([P, F, 16], bf16)
            nc.sync.dma_start(out=w1a_sb[:],
                              in_=w1ain[:].rearrange("p (k j) -> p k j", j=16))

            # ---- small resident tables ----
            ident = cpool.tile([P, P], f32)
            make_identity(nc, ident[:])
            identbf = cpool.tile([P, P], bf16)
            nc.vector.tensor_copy(out=identbf[:], in_=ident[:])
            onesbf = cpool.tile([P, 1], bf16)
            nc.vector.memset(onesbf[:], 1.0)
            esrc_sb = cpool.tile([P, NT1], i32)
            nc.sync.dma_start(out=esrc_sb[:], in_=esrc2d_d[:])
            dloc_sb = cpool.tile([P, NT1], f32)
            nc.sync.dma_start(out=dloc_sb[:], in_=dloc2d_d[:])
            iotar_sb = cpool.tile([P, DB], f32)
            nc.sync.dma_start(out=iotar_sb[:], in_=iotar[:])
            att2_sb = cpool.tile([P, F, 2], bf16)
            nc.sync.dma_start(out=att2_sb[:],
                              in_=att2sb[:].rearrange("p (m j) -> p m j", j=2))
            b1_sb = cpool.tile([P, 48], f32)
            nc.sync.dma_start(out=b1_sb[:], in_=b1in[:])
            b2bc_sb = cpool.tile([P, D], f32)
            nc.sync.dma_start(out=b2bc_sb[:], in_=b2bcin[:])
            colsm_sb = cpool.tile([P, F], f32)
            nc.sync.dma_start(out=colsm_sb[:], in_=colsmin[:])
            fcb_sb = cpool.tile([P, F], f32)
            nc.sync.dma_start(out=fcb_sb[:], in_=fcbin[:])
            clsb_sb = cpool.tile([2, 1], f32)
            nc.sync.dma_start(out=clsb_sb[:], in_=clsbin[:])
            mblk_sb = cpool.tile([DB, MB], i32)
            nc.sync.dma_start(out=mblk_sb[:], in_=mblk2d_d[:])
            esrc2m_sb = cpool.tile([P, NT2], i32)
            nc.sync.dma_start(out=esrc2m_sb[:], in_=esrc2m_d[:])
            dloc2m_sb = cpool.tile([P, NT2], f32)
            nc.sync.dma_start(out=dloc2m_sb[:], in_=dloc2m_d[:])
            mnode_sb = cpool.tile([P, MC // P], i32)
            nc.sync.dma_start(out=mnode_sb[:], in_=mnode2d_d[:])

            # ---- persistent big SBUF tensors ----
            h2t_sb = bigpool.tile([P, F, NACT], bf16)
            x2t_sb = bigpool.tile([P, 48, NACT], bf16)

            # ---- internal DRAM ----
            a1loc = dpool.tile([NSH, 16], bf16)
            a1_dram = dpool.tile([N, 16], bf16, addr_space="Shared")
            h2loc = dpool.tile([NACT, HW], bf16)
            a2loc = dpool.tile([NACT, 2], bf16)
            h2ag = dpool.tile([NCORES * NACT, HW], bf16, addr_space="Shared")

            with tc.tile_pool(name="aggp", bufs=1) as aggpool:
              aggT_sb = aggpool.tile([P, F, H1, NACT], fp8)
              with tc.tile_pool(name="xgp", bufs=1) as xgpool:
                # ====== X-row gathers for all layer-1 blocks, up-front ======
                xgall = xgpool.tile([P, NT1, D], bf16, name="xgall")
                for t in range(NT1):
                    nc.gpsimd.indirect_dma_start(
                        out=xgall[:, t, :], out_offset=None,
                        in_=Xbf[:],
                        in_offset=bass.IndirectOffsetOnAxis(
                            ap=esrc_sb[:, t:t + 1], axis=0))

                # ========= S2: local a1 = X_loc @ w1a, AllGather ============
                with tc.tile_pool(name="ps2", bufs=1, space="PSUM") as ps2:
                    a1r_sb = wpool.tile([P, NSH // P, 16], bf16, tag="a1r",
                                        bufs=1)
                    for ch in range(NSH // 512):
                        a1t_ps = ps2.tile([16, 512], f32, tag="a1t", bufs=2)
                        for kt in range(F):
                            nc.tensor.matmul(
                                out=a1t_ps[:], lhsT=w1a_sb[:, kt, :],
                                rhs=xt_sb[:, kt, ch * 512:(ch + 1) * 512],
                                start=(kt == 0), stop=(kt == F - 1))
                        a1t_sb = wpool.tile([16, 512], f32, tag="a1ts")
                        nc.vector.tensor_copy(out=a1t_sb[:], in_=a1t_ps[:])
                        for q in range(4):
                            tr_ps = ps2.tile([P, 16], f32, tag="tr16", bufs=2)
                            nc.tensor.transpose(
                                out=tr_ps[:],
                                in_=a1t_sb[:16, q * P:(q + 1) * P],
                                identity=ident[:16, :16])
                            nc.vector.tensor_copy(out=a1r_sb[:, ch * 4 + q, :],
                                                  in_=tr_ps[:])
                    nc.sync.dma_start(
                        out=a1loc[:].rearrange("(a p) b -> p a b", p=P),
                        in_=a1r_sb[:])
                    # dst-side scores for active nodes straight from XTact
                    a1blk_sb = cpool.tile([DB, NB, H1], bf16)
                    for ch in range(NC_CH):
                        aat_ps = ps2.tile([H1, 512], f32, tag="aat", bufs=2)
                        for kt in range(F):
                            nc.tensor.matmul(
                                out=aat_ps[:], lhsT=w1a_sb[:, kt, 8:16],
                                rhs=xta_sb[:, kt, ch * 512:(ch + 1) * 512],
                                start=(kt == 0), stop=(kt == F - 1))
                        aat_sb = wpool.tile([H1, 512], f32, tag="aats")
                        nc.vector.tensor_copy(out=aat_sb[:], in_=aat_ps[:])
                        for qq in range(8):
                            tra_ps = ps2.tile([DB, H1], f32, tag="tra", bufs=2)
                            nc.tensor.transpose(
                                out=tra_ps[:],
                                in_=aat_sb[:H1, qq * DB:(qq + 1) * DB],
                                identity=ident[:H1, :H1])
                            nc.vector.tensor_copy(out=a1blk_sb[:, ch * 8 + qq, :],
                                                  in_=tra_ps[:])
                nc.gpsimd.collective_compute(
                    "AllGather", mybir.AluOpType.bypass,
                    replica_groups=[CORE_IDS], ins=[a1loc[:]],
                    outs=[a1_dram[:]])
                if dbg:
                    nc.sync.dma_start(out=dbg_outs["dbg_a1"][:], in_=a1_dram[:])

                # ========= S3: layer-1 edge phase over active dst blocks ====
                with tc.tile_pool(name="ps3", bufs=1, space="PSUM") as ps3:
                    for b in range(NB):
                        g1blk = epool.tile([P, TB, 16], bf16, tag="g1", bufs=2)
                        for tt in range(TB):
                            nc.gpsimd.indirect_dma_start(
                                out=g1blk[:, tt, :], out_offset=None,
                                in_=a1_dram[:],
                                in_offset=bass.IndirectOffsetOnAxis(
                                    ap=esrc_sb[:, b * TB + tt:b * TB + tt + 1],
                                    axis=0))
                        a01 = epool.tile([P, TB, DB], bf16, tag="a01", bufs=2)
                        a01t = epool.tile([DB, TB, P], bf16, tag="a01t", bufs=2)
                        adb_ps = ps3.tile([P, TB, H1], f32, tag="adbre", bufs=2)
                        dn_ps = ps3.tile([DB, H1], f32, tag="dn", bufs=1)
                        for tt in range(TB):
                            t = b * TB + tt
                            nc.vector.tensor_scalar(
                                out=a01[:, tt, :], in0=iotar_sb[:],
                                scalar1=dloc_sb[:, t:t + 1], scalar2=None,
                                op0=mybir.AluOpType.is_equal)
                            trA_ps = ps3.tile([DB, P], bf16, tag="trA", bufs=2)
                            nc.tensor.transpose(out=trA_ps[:], in_=a01[:, tt, :],
                                                identity=identbf[:])
                            nc.vector.tensor_copy(out=a01t[:, tt, :],
                                                  in_=trA_ps[:])
                            nc.tensor.matmul(out=adb_ps[:, tt, :],
                                             lhsT=a01t[:, tt, :],
                                             rhs=a1blk_sb[:, b, :],
                                             start=True, stop=True)
                        s_sb = epool.tile([P, TB, H1], f32, tag="s", bufs=2)
                        nc.vector.tensor_tensor(out=s_sb[:],
                                                in0=g1blk[:, :, :H1],
                                                in1=adb_ps[:],
                                                op=mybir.AluOpType.add)
                        lk_sb = epool.tile([P, TB, H1], f32, tag="lk", bufs=2)
                        nc.vector.scalar_tensor_tensor(
                            out=lk_sb[:], in0=s_sb[:], scalar=NEG_SLOPE,
                            in1=s_sb[:], op0=mybir.AluOpType.mult,
                            op1=mybir.AluOpType.max)
                        p1_sb = epool.tile([P, TB, H1], bf16, tag="p1", bufs=2)
                        nc.scalar.activation(
                            out=p1_sb[:], in_=lk_sb[:],
                            func=mybir.ActivationFunctionType.Exp)
                        for tt in range(TB):
                            nc.tensor.matmul(out=dn_ps[:], lhsT=a01[:, tt, :],
                                             rhs=p1_sb[:, tt, :],
                                             start=(tt == 0),
                                             stop=(tt == TB - 1))
                        # pad dst slots have no edges: dn=0 would give inf
                        # and 0*inf=NaN in the broadcast matmul below
                        dneps = wpool.tile([DB, H1], f32, tag="dneps")
                        nc.vector.tensor_scalar(out=dneps[:], in0=dn_ps[:],
                                                scalar1=1e-20, scalar2=None,
                                                op0=mybir.AluOpType.add)
                        recip_sb = wpool.tile([DB, H1], bf16, tag="recip")
                        with nc.allow_low_precision(
                                reason="softmax denom in bf16"):
                            nc.vector.reciprocal(out=recip_sb[:], in_=dneps[:])
                        reb_ps = ps3.tile([P, TB, H1], f32, tag="adbre", bufs=2)
                        for tt in range(TB):
                            nc.tensor.matmul(out=reb_ps[:, tt, :],
                                             lhsT=a01t[:, tt, :],
                                             rhs=recip_sb[:],
                                             start=True, stop=True)
                        alphab = epool.tile([P, TB, H1], f32, tag="alpha",
                                            bufs=2)
                        nc.vector.tensor_tensor(
                            out=alphab[:], in0=p1_sb[:],
                            in1=reb_ps[:], op=mybir.AluOpType.mult)
                        aalls = []
                        for tt in range(TB):
                            a_all = epool.tile([P, H1 * DB], bf16, tag="aall",
                                               bufs=TB + 1)
                            for h in range(H1):
                                if h % 2 == 0:
                                    nc.vector.tensor_scalar(
                                        out=a_all[:, h * DB:(h + 1) * DB],
                                        in0=a01[:, tt, :],
                                        scalar1=alphab[:, tt, h:h + 1],
                                        scalar2=None,
                                        op0=mybir.AluOpType.mult)
                                else:
                                    nc.scalar.activation(
                                        out=a_all[:, h * DB:(h + 1) * DB],
                                        in_=a01[:, tt, :],
                                        func=mybir.ActivationFunctionType.Copy,
                                        scale=alphab[:, tt, h:h + 1])
                            aalls.append(a_all)
                        for fs in range(2):
                            for f in range(fs * 3, fs * 3 + 3):
                                ag_ps = ps3.tile([P, H1 * DB], f32,
                                                 tag=f"agg{f % 3}", bufs=1)
                                for tt in range(TB):
                                    nc.tensor.matmul(
                                        out=ag_ps[:],
                                        lhsT=xgall[:, b * TB + tt,
                                                   f * P:(f + 1) * P],
                                        rhs=aalls[tt][:],
                                        start=(tt == 0), stop=(tt == TB - 1))
                                agdst = aggT_sb[:, f, :, b * DB:(b + 1) * DB]
                                agsrc = ag_ps[:].rearrange(
                                    "p (h d) -> p h d", h=H1)
                                if f % 2 == 1:
                                    nc.scalar.activation(
                                        out=agdst, in_=agsrc,
                                        func=mybir.ActivationFunctionType.Copy)
                                else:
                                    nc.vector.tensor_copy(out=agdst, in_=agsrc)
              if dbg:
                  for f in range(F):
                      for hh in range(H1):
                          agf = wpool.tile([P, NACT], f32, tag="dbgxf")
                          nc.vector.tensor_copy(out=agf[:],
                                                in_=aggT_sb[:, f, hh, :])
                          nc.sync.dma_start(
                              out=dbg_outs["dbg_agg"]
                              [f * P:(f + 1) * P,
                               hh * NACT:(hh + 1) * NACT],
                              in_=agf[:])
              # ======= S4: x2 = elu(aggT @ W1 + b1) + 1, SBUF-resident ======
              inv_s4 = 1.0 / WSCALE
              with tc.tile_pool(name="ps4", bufs=1, space="PSUM") as ps4:
                for h in range(H1):
                    for m in range(F):
                        j = h * F + m
                        w1hm = spool.tile([P, F, P], fp8, tag="w1hm", bufs=3)
                        nc.sync.dma_start(
                            out=w1hm[:],
                            in_=W1r[:].rearrange("p (j k q) -> p j k q",
                                                 j=48, k=F)[:, j, :, :])
                        for cc in range(NC_CH):
                            o1_ps = ps4.tile([P, 512], f32, tag="o1", bufs=2)
                            for k2 in range(F // 2):
                                nc.tensor.matmul(
                                    out=o1_ps[:],
                                    lhsT=w1hm[:, 2 * k2:2 * k2 + 2, :],
                                    rhs=aggT_sb[:, 2 * k2:2 * k2 + 2, h,
                                                cc * 512:(cc + 1) * 512],
                                    start=(k2 == 0), stop=(k2 == F // 2 - 1),
                                    perf_mode=mybir.MatmulPerfMode.DoubleRow)
                            ebuf = wpool.tile([P, 512], bf16, tag="ebuf")
                            nc.scalar.activation(
                                out=ebuf[:], in_=o1_ps[:],
                                func=mybir.ActivationFunctionType.Exp,
                                bias=b1_sb[:, j:j + 1], scale=inv_s4)
                            t1a = wpool.tile([P, 512], bf16, tag="t1a")
                            if j % 2 == 0:
                                nc.scalar.activation(
                                    out=t1a[:], in_=o1_ps[:],
                                    func=mybir.ActivationFunctionType.Relu,
                                    bias=b1_sb[:, j:j + 1], scale=inv_s4)
                            else:
                                xb = wpool.tile([P, 512], f32, tag="xb")
                                nc.vector.tensor_scalar(
                                    out=xb[:], in0=o1_ps[:],
                                    scalar1=inv_s4,
                                    scalar2=b1_sb[:, j:j + 1],
                                    op0=mybir.AluOpType.mult,
                                    op1=mybir.AluOpType.add)
                                nc.vector.tensor_scalar(
                                    out=t1a[:], in0=xb[:], scalar1=0.0,
                                    scalar2=None, op0=mybir.AluOpType.max)
                            # x2' = x2 + 1 = relu(xb) + min(exp(xb), 1); the +1
                            # is compensated via colsum(W2) subtraction from h2
                            nc.vector.scalar_tensor_tensor(
                                out=x2t_sb[:, j, cc * 512:(cc + 1) * 512],
                                in0=ebuf[:], scalar=1.0, in1=t1a[:],
                                op0=mybir.AluOpType.min,
                                op1=mybir.AluOpType.add)
            if dbg:
                for kt in range(48):
                    xxf = wpool.tile([P, NACT], f32, tag="dbgxf")
                    nc.vector.tensor_copy(out=xxf[:], in_=x2t_sb[:, kt, :])
                    nc.sync.dma_start(
                        out=dbg_outs["dbg_x2t"][kt * P:(kt + 1) * P, :],
                        in_=xxf[:])

            # ---- tail-phase SBUF tensors (after layer-1 pools are freed) ---
            with tc.tile_pool(name="tailp", bufs=1) as tailpool:
                # x_emb gathers early (overlap S5 on gpsimd)
                xem_r = tailpool.tile([P, MC // P, D], bf16, name="xem_r")
                for q in range(MC // P):
                    nc.gpsimd.indirect_dma_start(
                        out=xem_r[:, q, :], out_offset=None, in_=Xbf[:],
                        in_offset=bass.IndirectOffsetOnAxis(
                            ap=mnode_sb[:, q:q + 1], axis=0))

                # ========= S5: h2 = x2' @ W2 - colsum, a2, h2 rows ==========
                h2n_sb = tailpool.tile([P, NACT // P, HW], bf16, name="h2n_sb")
                with tc.tile_pool(name="ps5", bufs=1, space="PSUM") as ps5:
                    for cc in range(NC_CH):
                        h2_ps = [ps5.tile([P, 512], f32, tag=f"h2_{m}", bufs=1,
                                          name=f"h2ps{m}")
                                 for m in range(F)]
                        for kt in range(48):
                            w2kt = spool.tile([P, D], bf16, tag="w2kt", bufs=4)
                            nc.sync.dma_start(
                                out=w2kt[:],
                                in_=W2r[:].rearrange("p (k n) -> p k n",
                                                     n=D)[:, kt, :])
                            for m in range(F):
                                nc.tensor.matmul(
                                    out=h2_ps[m][:],
                                    lhsT=w2kt[:, m * P:(m + 1) * P],
                                    rhs=x2t_sb[:, kt, cc * 512:(cc + 1) * 512],
                                    start=(kt == 0), stop=(kt == 47))
                        for m in range(F):
                            nc.vector.tensor_scalar(
                                out=h2t_sb[:, m, cc * 512:(cc + 1) * 512],
                                in0=h2_ps[m][:],
                                scalar1=colsm_sb[:, m:m + 1], scalar2=None,
                                op0=mybir.AluOpType.subtract)
                        for q4 in range(4):
                            q = cc * 4 + q4
                            c0 = cc * 512 + q4 * P
                            for m in range(F):
                                trh_ps = ps5.tile([P, P], bf16, tag="trh",
                                                  bufs=1)
                                nc.tensor.transpose(
                                    out=trh_ps[:],
                                    in_=h2t_sb[:, m, c0:c0 + P],
                                    identity=identbf[:])
                                nc.vector.tensor_copy(
                                    out=h2n_sb[:, q, m * P:(m + 1) * P],
                                    in_=trh_ps[:])
                            a2_ps = ps5.tile([P, 2], f32, tag="a2", bufs=1)
                            for m in range(F):
                                nc.tensor.matmul(out=a2_ps[:],
                                                 lhsT=h2t_sb[:, m, c0:c0 + P],
                                                 rhs=att2_sb[:, m, :],
                                                 start=(m == 0),
                                                 stop=(m == F - 1))
                            a2b = wpool.tile([P, 2], bf16, tag="a2b")
                            nc.vector.tensor_copy(out=a2b[:], in_=a2_ps[:])
                            nc.vector.tensor_copy(out=h2n_sb[:, q, D:D + 2],
                                                  in_=a2b[:])
                            nc.vector.tensor_copy(
                                out=h2n_sb[:, q, D + 2:D + 3], in_=onesbf[:])
                            nc.sync.dma_start(out=a2loc[q * P:(q + 1) * P, :],
                                              in_=a2b[:])
                nc.sync.dma_start(
                    out=h2loc[:].rearrange("(a p) b -> p a b", p=P),
                    in_=h2n_sb[:])
                if dbg:
                    for q in range(NACT // P):
                        h2f = wpool.tile([P, HW], f32, tag="dbgxf")
                        nc.vector.tensor_copy(out=h2f[:], in_=h2n_sb[:, q, :])
                        nc.sync.dma_start(
                            out=dbg_outs["dbg_h2"][q * P:(q + 1) * P, :],
                            in_=h2f[:])

                # a2 dst rows of the masked slots (gathered by active slot)
                a2blk_sb = cpool.tile([DB, MB, 2], bf16)
                for b2 in range(MB):
                    nc.gpsimd.indirect_dma_start(
                        out=a2blk_sb[:, b2, :], out_offset=None, in_=a2loc[:],
                        in_offset=bass.IndirectOffsetOnAxis(
                            ap=mblk_sb[:, b2:b2 + 1], axis=0))

                # ===== S7 prep + S8 prep (overlap the h2 AllGather) =========
                with tc.tile_pool(name="ps8", bufs=1, space="PSUM") as ps8:
                    # x_emb transposes -> feature-major
                    xem_bf = tailpool.tile([P, F, MC], bf16)
                    for q in range(MC // P):
                        for f in range(F):
                            tre_ps = ps8.tile([P, P], bf16, tag="tr", bufs=2)
                            nc.tensor.transpose(
                                out=tre_ps[:],
                                in_=xem_r[:, q, f * P:(f + 1) * P],
                                identity=identbf[:])
                            nc.vector.tensor_copy(
                                out=xem_bf[:, f, q * P:(q + 1) * P],
                                in_=tre_ps[:])
                    # fc partial accumulation over the x_emb half (kt 6..11)
                    fcw_sb = tailpool.tile([P, F, 12, P], bf16)
                    for m in range(F):
                        nc.sync.dma_start(
                            out=fcw_sb[:, m, :, :],
                            in_=fcwr[:].rearrange("p (m k q) -> p m k q",
                                                  m=F, k=12)[:, m, :, :])
                    clsw_sb = cpool.tile([P, F, 2], bf16)
                    nc.sync.dma_start(
                        out=clsw_sb[:],
                        in_=clswbf[:].rearrange("(m p) n -> p m n", p=P))
                    # x_emb half of fc, accumulated to SBUF (psum banks are
                    # needed by S7 while the collective runs)
                    fcacc = tailpool.tile([P, F, MC], f32, name="fcacc")
                    for m in range(F):
                        fcA_ps = ps8.tile([P, MC], f32, tag="fcA", bufs=2)
                        for kt in range(6, 12):
                            nc.tensor.matmul(out=fcA_ps[:],
                                             lhsT=fcw_sb[:, m, kt, :],
                                             rhs=xem_bf[:, kt - F, :],
                                             start=(kt == 6), stop=(kt == 11))
                        nc.vector.tensor_scalar(out=fcacc[:, m, :],
                                                in0=fcA_ps[:],
                                                scalar1=fcb_sb[:, m:m + 1],
                                                scalar2=None,
                                                op0=mybir.AluOpType.add)

                    # layer-2 one-hots + dst scores (independent of collective)
                    a01m = tailpool.tile([P, NT2, DB], bf16, name="a01m")
                    a01tm = tailpool.tile([DB, NT2, P], bf16, name="a01tm")
                    ad2b = tailpool.tile([P, NT2, 1], f32, name="ad2b")
                    for b2 in range(MB):
                        for tt in range(TB2):
                            t = b2 * TB2 + tt
                            nc.vector.tensor_scalar(
                                out=a01m[:, t, :], in0=iotar_sb[:],
                                scalar1=dloc2m_sb[:, t:t + 1], scalar2=None,
                                op0=mybir.AluOpType.is_equal)
                            trm_ps = ps8.tile([DB, P], bf16, tag="tr", bufs=2)
                            nc.tensor.transpose(out=trm_ps[:],
                                                in_=a01m[:, t, :],
                                                identity=identbf[:])
                            nc.vector.tensor_copy(out=a01tm[:, t, :],
                                                  in_=trm_ps[:])
                            ad2_ps = ps8.tile([P, 1], f32, tag="tr", bufs=2)
                            nc.tensor.matmul(out=ad2_ps[:],
                                             lhsT=a01tm[:, t, :],
                                             rhs=a2blk_sb[:, b2, 1:2],
                                             start=True, stop=True)
                            nc.vector.tensor_copy(out=ad2b[:, t, :],
                                                  in_=ad2_ps[:])

                    # ========= S6: AllGather h2 rows ========================
                    nc.gpsimd.collective_compute(
                        "AllGather", mybir.AluOpType.bypass,
                        replica_groups=[CORE_IDS], ins=[h2loc[:]],
                        outs=[h2ag[:]])

                    # ========= S7: layer-2 edge phase, masked dst only ======
                    xgm_bf = tailpool.tile([P, F, MC], bf16)
                    for b2 in range(MB):
                        outA_ps = ps8.tile([DB, 512], f32, tag="outA", bufs=1)
                        outB_ps = ps8.tile([DB, HW - 512], f32, tag="outB",
                                           bufs=1)
                        hgblk = epool.tile([P, TB2, HW], bf16, tag="hg",
                                           bufs=2)
                        for tt in range(TB2):
                            t2 = b2 * TB2 + tt
                            nc.gpsimd.indirect_dma_start(
                                out=hgblk[:, tt, :], out_offset=None,
                                in_=h2ag[:],
                                in_offset=bass.IndirectOffsetOnAxis(
                                    ap=esrc2m_sb[:, t2:t2 + 1], axis=0))
                        s_sb = epool.tile([P, TB2, 1], f32, tag="s2", bufs=2)
                        nc.vector.tensor_tensor(
                            out=s_sb[:], in0=hgblk[:, :, D:D + 1],
                            in1=ad2b[:, b2 * TB2:(b2 + 1) * TB2, :],
                            op=mybir.AluOpType.add)
                        lk_sb = epool.tile([P, TB2, 1], f32, tag="lk2", bufs=2)
                        nc.vector.scalar_tensor_tensor(
                            out=lk_sb[:], in0=s_sb[:], scalar=NEG_SLOPE,
                            in1=s_sb[:], op0=mybir.AluOpType.mult,
                            op1=mybir.AluOpType.max)
                        p2_sb = epool.tile([P, TB2, 1], f32, tag="p2", bufs=2)
                        nc.scalar.activation(
                            out=p2_sb[:], in_=lk_sb[:],
                            func=mybir.ActivationFunctionType.Exp)
                        for tt in range(TB2):
                            t = b2 * TB2 + tt
                            a_all = epool.tile([P, DB], bf16, tag="aall2")
                            if tt % 2 == 0:
                                nc.vector.tensor_scalar(
                                    out=a_all[:], in0=a01m[:, t, :],
                                    scalar1=p2_sb[:, tt, 0:1], scalar2=None,
                                    op0=mybir.AluOpType.mult)
                            else:
                                nc.scalar.activation(
                                    out=a_all[:], in_=a01m[:, t, :],
                                    func=mybir.ActivationFunctionType.Copy,
                                    scale=p2_sb[:, tt, 0:1])
                            nc.tensor.matmul(out=outA_ps[:], lhsT=a_all[:],
                                             rhs=hgblk[:, tt, 0:512],
                                             start=(tt == 0),
                                             stop=(tt == TB2 - 1))
                            nc.tensor.matmul(out=outB_ps[:], lhsT=a_all[:],
                                             rhs=hgblk[:, tt, 512:HW],
                                             start=(tt == 0),
                                             stop=(tt == TB2 - 1))
                        recd_sb = wpool.tile([DB, 1], f32, tag="recd")
                        nc.vector.reciprocal(
                            out=recd_sb[:],
                            in_=outB_ps[:, D + 2 - 512:D + 3 - 512])
                        o2_sb = wpool.tile([DB, D], f32, tag="o2sb")
                        nc.vector.scalar_tensor_tensor(
                            out=o2_sb[:, 0:512], in0=outA_ps[:],
                            scalar=recd_sb[:, 0:1], in1=b2bc_sb[:DB, 0:512],
                            op0=mybir.AluOpType.mult, op1=mybir.AluOpType.add)
                        nc.vector.scalar_tensor_tensor(
                            out=o2_sb[:, 512:D], in0=outB_ps[:, 0:D - 512],
                            scalar=recd_sb[:, 0:1], in1=b2bc_sb[:DB, 512:D],
                            op0=mybir.AluOpType.mult, op1=mybir.AluOpType.add)
                        if dbg:
                            nc.sync.dma_start(
                                out=dbg_outs["dbg_o2"][b2 * DB:(b2 + 1) * DB, :],
                                in_=o2_sb[:])
                        for f in range(F):
                            tro_ps = ps8.tile([P, DB], f32, tag="tr", bufs=2)
                            nc.tensor.transpose(
                                out=tro_ps[:],
                                in_=o2_sb[:, f * P:(f + 1) * P],
                                identity=ident[:DB, :DB])
                            nc.vector.tensor_copy(
                                out=xgm_bf[:, f, b2 * DB:(b2 + 1) * DB],
                                in_=tro_ps[:])

                    # ===== S8: finish fc (x_gemb half) + classifier =========
                    fcT_bf = tailpool.tile([P, F, MC], bf16)
                    for m in range(F):
                        fcB_ps = ps8.tile([P, MC], f32, tag="fcA", bufs=2)
                        for kt in range(6):
                            nc.tensor.matmul(out=fcB_ps[:],
                                             lhsT=fcw_sb[:, m, kt, :],
                                             rhs=xgm_bf[:, kt, :],
                                             start=(kt == 0), stop=(kt == 5))
                        nc.vector.tensor_tensor(out=fcT_bf[:, m, :],
                                                in0=fcB_ps[:],
                                                in1=fcacc[:, m, :],
                                                op=mybir.AluOpType.add)
                    cls_ps = ps8.tile([2, MC], f32, tag="cls", bufs=1)
                    for m in range(F):
                        nc.tensor.matmul(out=cls_ps[:], lhsT=clsw_sb[:, m, :],
                                         rhs=fcT_bf[:, m, :],
                                         start=(m == 0), stop=(m == F - 1))
                    outf = wpool.tile([2, MC], f32, tag="outf")
                    nc.vector.tensor_scalar(out=outf[:], in0=cls_ps[:],
                                            scalar1=clsb_sb[:, 0:1],
                                            scalar2=None,
                                            op0=mybir.AluOpType.add)
                    nc.sync.dma_start(out=out_t[:], in_=outf[:])

    _split_excess_waits(nc)
    return nc


# ---------------------------------------------------------------------------
def kernel(cls_embeddings, edge_index, mask_idx, W1, att_src1, att_dst1, b1,
           W2, att_src2, att_dst2, b2, fc_w, fc_b, cls_w, cls_b, _dbg=False):
    X = np.asarray(cls_embeddings, dtype=np.float32)
    per_core, positions, act_lists, NACT, TB, MC, TB2 = _preprocess(
        np.asarray(edge_index), np.asarray(mask_idx))

    # host-folded attention basis: w1a[d, j] = sum_c W1[d, hc] att_j[h, c]
    W1f = np.asarray(W1, np.float32).reshape(D, H1, D)
    w1a = np.concatenate(
        [np.einsum("dhc,hc->dh", W1f, np.asarray(att_src1, np.float32)),
         np.einsum("dhc,hc->dh", W1f, np.asarray(att_dst1, np.float32))],
        axis=1)                                    # [768, 16]
    w1ain = np.ascontiguousarray(
        w1a.reshape(F, P, 16).transpose(1, 0, 2).reshape(P, F * 16))

    att2T = np.stack([np.asarray(att_src2, np.float32)[0],
                      np.asarray(att_dst2, np.float32)[0]], axis=1)
    att2_sb = np.ascontiguousarray(
        att2T.reshape(F, P, 2).transpose(1, 0, 2).reshape(P, F * 2))

    XT = np.ascontiguousarray(X.T).astype(BF)      # [768, 8192]

    W1h = np.asarray(W1, np.float32)
    W2h = np.asarray(W2, np.float32)
    # W1r[p, j, kt, q] = W1[kt*128+p, j*128+q] * WSCALE  (fp8)
    W1r = np.ascontiguousarray(
        (W1h * WSCALE).reshape(F, P, 48, P).transpose(1, 2, 0, 3)
        .reshape(P, 48 * F * P)).astype(F8)
    # W2r[p, kt, n] = W2[kt*128+p, n]
    W2r = np.ascontiguousarray(
        W2h.reshape(48, P, D).transpose(1, 0, 2).reshape(P, 48 * D)).astype(BF)
    # fcwr[p, m, kt, q] = fc_w[kt*128+p, m*128+q]
    fch = np.asarray(fc_w, np.float32)
    fcwr = np.ascontiguousarray(
        fch.reshape(12, P, F, P).transpose(1, 2, 0, 3)
        .reshape(P, F * 12 * P)).astype(BF)

    shared = {
        "Xbf": X.astype(BF),
        "w1ain": w1ain.astype(BF),
        "W1r": W1r,
        "W2r": W2r,
        "att2sb": att2_sb.astype(BF),
        "fcwr": fcwr,
        "clswbf": np.asarray(cls_w, np.float32).astype(BF),
        "b1in": np.ascontiguousarray(np.asarray(b1, np.float32).reshape(48, P).T),
        "b2bcin": np.tile(np.asarray(b2, np.float32).reshape(1, D), (P, 1)),
        "colsmin": np.ascontiguousarray(
            W2h.sum(axis=0, dtype=np.float64).astype(np.float32)
            .reshape(F, P).T),
        "fcbin": np.ascontiguousarray(np.asarray(fc_b, np.float32).reshape(F, P).T),
        "clsbin": np.asarray(cls_b, np.float32).reshape(2, 1),
        "iotar": np.tile(np.arange(DB, dtype=np.float32), (P, 1)),
    }

    nc = _build_program(NACT, TB, MC, TB2, dbg=_dbg)
    in_maps = []
    for c in range(NCORES):
        m = dict(shared)
        m.update(per_core[c])
        xtl = XT[:, c * NSH:(c + 1) * NSH]
        m["XTloc"] = np.ascontiguousarray(
            xtl.reshape(F, P, NSH).transpose(1, 0, 2))
        xta = np.zeros((D, NACT), dtype=BF)
        xta[:, :len(act_lists[c])] = XT[:, act_lists[c]]
        m["XTact"] = np.ascontiguousarray(
            xta.reshape(F, P, NACT).transpose(1, 0, 2))
        in_maps.append(m)

    global LAST
    kres = run_bass_kernel_spmd(nc, in_maps, list(range(NCORES)),
                                trace=TRACE, tmpdir=TRACE_DIR)
    LAST = kres
    res = kres.results

    out = np.zeros((M, 2), dtype=np.float32)
    for c in range(NCORES):
        pos = positions[c]
        ot = res[c]["out_t"]
        for j, p_ in enumerate(pos):
            out[p_] = ot[:, j]
    if _dbg:
        return out, res, positions, act_lists
    return out


# revision 16
# speedup vs baseline: 2.0441x; 1.0708x over previous
"""Two-layer GAT (PyG-style GATConv) on 8 Trainium2 NeuronCores.

v4 layout (active-set, SBUF-resident intermediates, dma_gather):
- Only "active" nodes (srcs of edges into masked dsts, plus masked dsts)
  need layer-1 output / layer-2 features: ~45% of all nodes. Layer-1
  aggregation, S4 (agg@W1), S5 (x2@W2) and the h2 exchange are restricted
  to the per-core active set (padded to NACT=512 vs 1024 owned nodes).
- Single S3 pass over all dst blocks -> aggT (fp8) fully SBUF-resident;
  S4 streams W1 once, keeps x2t in SBUF (no DRAM round-trip); S5 streams
  W2 once against the SBUF-resident x2t.
- All row gathers (X rows per edge, a1 rows per edge, h2 rows per
  layer-2 edge, x_emb rows) are single dma_gather instructions (SWDGE
  descriptor generation is ~1us + 0.34ns/row; per-tile indirect DMAs
  were the previous bottleneck). a1/h2 rows are padded to 256B multiples
  to satisfy the gather's element-size constraint.
- S3 one-hot/transpose/dst-score prep for all tiles is hoisted before the
  a1 AllGather; per-edge scores are computed in 3 batched ops; the
  alpha-scaled one-hot uses a single broadcast tensor_tensor per tile.
- h2 AllGather carries only active rows ([NACT, 896] per core); S8 prep
  (x_emb transposes, fc weight loads, fc partial accumulation) overlaps
  the collective.
- Masked fc/classifier on the owning core; host reassembles the output.
"""
import numpy as np
from concourse import bass, mybir
import concourse.tile as tile
from concourse.bass_utils import run_bass_kernel_spmd
from concourse.vector_clock import ScopedClock, VectorClock
from concourse.masks import make_identity

N, E, M = 8192, 32768, 1024
D = 768
F = 6               # 768 / 128
H1 = 8
NCORES = 8
NSH = N // NCORES   # 1024 nodes owned per core
DB = 64
P = 128
HW = D + 4          # h2 row: 768 h2 | a_s2 | a_d2 | 1.0 | pad
AW = 16             # a1 row: 8 a_src | 8 a_dst

f32 = mybir.dt.float32
bf16 = mybir.dt.bfloat16
fp8 = mybir.dt.float8e4
i32 = mybir.dt.int32
i16 = mybir.dt.int16
BF = mybir.dt.np(bf16)
F8 = mybir.dt.np(fp8)
WSCALE = 32.0       # fp8 W1 pre-scale (undone via activation scale)

NEG_SLOPE = 0.2

# test-harness knobs (harness calls kernel() with defaults: no tracing)
TRACE = False
TRACE_DIR = None
LAST = None


# ---------------------------------------------------------------------------
# The walrus build in this container rejects a Drain instruction with more
# than one semaphore wait ("Too many sync wait commands"); the default
# TileContext kernel-tail drain has many. Emit one single-wait drain per
# logical processor instead.
def _split_drain_and_barrier(self, tick_clock, wait_clock):
    gc = tick_clock.global_clock
    nprocs = 27
    for i in range(nprocs):
        mask = [0] * nprocs
        mask[i] = 1 << 30
        part = gc.elementwise_min(VectorClock(mask))
        d = self.nc.sync.drain()
        wait_clock.add_sem_waits(d.ins, ScopedClock({None: part}))
    self.nc.all_engine_barrier()
    popped = self.nc._tile_sem_poison_stack.pop()
    assert popped is self._sem_poison
    self.nc.clear_and_free_semaphores(list(self.sems.allocated().values()))
    self.nc.all_engine_barrier()


tile.TileContext._drain_and_barrier = _split_drain_and_barrier

MAX_WAITS = 1  # this walrus build rejects multi-sem-wait instructions


def _split_excess_waits(nc):
    """Move excess semaphore waits onto preceding same-engine NoOps."""
    n_split = 0
    for bb in nc.m.functions[0].blocks:
        insts = bb.instructions
        idx = 0
        while idx < len(insts):
            inst = insts[idx]
            si = inst.sync_info
            if si is not None and len(si.on_wait) > MAX_WAITS:
                waits = list(si.on_wait)
                keep = waits[-MAX_WAITS:]
                extra = waits[:-MAX_WAITS]
                for gi in range(0, len(extra), MAX_WAITS):
                    nop = mybir.InstNoOp(
                        name=f"WSPLIT-{nc.next_id()}",
                        sync_info=mybir.SyncInfo(
                            on_wait=extra[gi:gi + MAX_WAITS], on_update=[]),
                        bass_nofuse=True,
                        engine=inst.engine,
                        ins=[], outs=[],
                    )
                    nc.register_instruction(nop)
                    insts.insert(idx, nop)
                    idx += 1
                inst.sync_info = mybir.SyncInfo(
                    on_wait=keep, on_update=list(si.on_update))
                n_split += 1
            idx += 1
    return n_split


def _bcast(ap, pos, n):
    """Insert a stride-0 broadcast dim of size n at free-dim position pos."""
    layout = [list(d) for d in ap.ap]
    layout.insert(pos, [0, n])
    return bass.AP(ap.tensor, ap.offset, layout)


def _wrap16(flat):
    """dma_gather index table: [128, n/16] i16, idx i at [i%16, i//16],
    replicated across the 8 Q7 cores (partition groups of 16)."""
    t = np.asarray(flat, dtype=np.int16).reshape(-1, 16).T
    return np.ascontiguousarray(np.tile(t, (8, 1)))


# ---------------------------------------------------------------------------
def _preprocess(edge_index, mask_idx):
    """Host-side graph partitioning: integer index work only."""
    src = np.asarray(edge_index[0], dtype=np.int64)
    dst = np.asarray(edge_index[1], dtype=np.int64)
    loop = np.arange(N, dtype=np.int64)
    src = np.concatenate([src, loop])
    dst = np.concatenate([dst, loop])
    mask = np.asarray(mask_idx, dtype=np.int64)

    # active set: masked nodes + srcs of edges into masked nodes
    mset = np.zeros(N, dtype=bool)
    mset[mask] = True
    need = np.zeros(N, dtype=bool)
    need[src[mset[dst]]] = True
    need[mask] = True

    aslot = np.full(N, -1, dtype=np.int64)
    act_lists = []
    for c in range(NCORES):
        nodes = np.nonzero(need[c * NSH:(c + 1) * NSH])[0] + c * NSH
        act_lists.append(nodes)
        aslot[nodes] = np.arange(len(nodes))
    NACT = int(np.ceil(max(len(a) for a in act_lists) / 512)) * 512
    NB = NACT // DB
    gid = (np.arange(N) // NSH) * NACT + aslot   # row in h2ag (valid if need)

    # ---- layer-1 edges: those into active dsts, bucketed (core, block) ----
    keep = need[dst]
    e_src, e_dst = src[keep], dst[keep]
    e_core = e_dst // NSH
    e_slot = aslot[e_dst]
    bucket = e_core * NB + e_slot // DB
    counts = np.bincount(bucket, minlength=NCORES * NB)
    TB = int(np.ceil(counts.max() / P))
    NT1 = NB * TB
    order = np.argsort(bucket, kind='stable')
    starts = np.zeros(NCORES * NB + 1, dtype=np.int64)
    np.cumsum(counts, out=starts[1:])
    pos = np.arange(len(order)) - starts[bucket[order]]
    flat = np.zeros((NCORES, NB * TB * P), dtype=np.int64)      # src node ids
    dflat = np.full((NCORES, NB * TB * P), 1000.0, dtype=np.float32)
    bo = bucket[order]
    addr = (bo % NB) * TB * P + pos
    flat[e_core[order], addr] = e_src[order]
    dflat[e_core[order], addr] = (e_slot[order] % DB).astype(np.float32)

    def to2d(a, nt):
        return np.ascontiguousarray(a.reshape(nt, P).T)

    per_core = []
    for c in range(NCORES):
        per_core.append(dict(
            esrc2d=to2d(flat[c].astype(np.int32), NT1),
            dloc2d=to2d(dflat[c], NT1),
        ))

    # ---- masked nodes per owning core ----
    positions = [np.nonzero(mask // NSH == c)[0] for c in range(NCORES)]
    MC = max(P, int(np.ceil(max(len(p) for p in positions) / P)) * P)
    MB = MC // DB

    # layer-2 edges grouped per masked occurrence (src is always active)
    l2 = mset[dst]
    l2_src, l2_dst = src[l2], dst[l2]
    d_order = np.argsort(l2_dst, kind='stable')
    sd = l2_dst[d_order]
    cnt2 = np.zeros((NCORES, MB), dtype=np.int64)
    for c in range(NCORES):
        pos_c = positions[c]
        nodes = mask[pos_c]
        lo = np.searchsorted(sd, nodes, side='left')
        hi = np.searchsorted(sd, nodes, side='right')
        deg = hi - lo
        nb2 = np.minimum(np.arange(len(pos_c)) // DB, MB - 1)
        np.add.at(cnt2[c], nb2, deg)
    TB2 = int(np.ceil(cnt2.max() / P))
    NT2 = MB * TB2
    for c in range(NCORES):
        pos_c = positions[c]
        nodes = mask[pos_c]
        mnode = np.zeros(MC, dtype=np.int64)
        mnode[:len(pos_c)] = nodes
        mloc = np.zeros(MC, dtype=np.int64)
        mloc[:len(pos_c)] = aslot[nodes]
        per_core[c]["mnode2d"] = np.ascontiguousarray(
            mnode.reshape(MC // P, P).T.astype(np.int32))
        per_core[c]["mblk2d"] = np.ascontiguousarray(
            mloc.reshape(MB, DB).T.astype(np.int32))
        esrc2 = np.zeros(NT2 * P, dtype=np.int64)
        dloc2 = np.full(NT2 * P, 1000.0, dtype=np.float32)
        lo = np.searchsorted(sd, nodes, side='left')
        hi = np.searchsorted(sd, nodes, side='right')
        for b2 in range(MB):
            base = b2 * TB2 * P
            k = 0
            for j in range(b2 * DB, min((b2 + 1) * DB, len(pos_c))):
                for e in d_order[lo[j]:hi[j]]:
                    esrc2[base + k] = gid[l2_src[e]]
                    dloc2[base + k] = j % DB
                    k += 1
        per_core[c]["esrc2m"] = to2d(esrc2.astype(np.int32), NT2)
        per_core[c]["dloc2m"] = to2d(dloc2, NT2)
    return per_core, positions, act_lists, NACT, TB, MC, TB2


# ---------------------------------------------------------------------------
def _build_program(NACT, TB, MC, TB2, dbg=False):
    NB = NACT // DB
    NC_CH = NACT // 512          # 512-wide chunks of the active set
    MB = MC // DB
    NT1 = NB * TB
    NT2 = MB * TB2
    nc = bass.Bass("TRN2", target_bir_lowering=False, debug=False,
                   num_devices=NCORES)
    dp = lambda name, shape, dt: nc.declare_dram_parameter(
        name, list(shape), dt, isOutput=False)

    Xbf = dp("Xbf", [N, D], bf16)
    XTloc = dp("XTloc", [P, F, NSH], bf16)           # per-core X.T (own nodes)
    XTact = dp("XTact", [P, F, NACT], bf16)          # per-core X.T (active)
    w1ain = dp("w1ain", [P, F * 16], bf16)           # host-folded att basis
    W1r = dp("W1r", [P, 48 * F * P], fp8)            # [p, j, kt, q] fp8*WSCALE
    W2r = dp("W2r", [P, 48 * D], bf16)               # [p, kt, n]
    att2sb = dp("att2sb", [P, F * 2], bf16)
    fcwr = dp("fcwr", [P, F * 12 * P], bf16)         # [p, m, kt, q]
    clswbf = dp("clswbf", [D, 2], bf16)
    b1in = dp("b1in", [P, 48], f32)
    b2bcin = dp("b2bcin", [P, D], f32)
    colsmin = dp("colsmin", [P, F], f32)
    fcbin = dp("fcbin", [P, F], f32)
    clsbin = dp("clsbin", [2, 1], f32)
    iotar = dp("iotar", [P, DB], f32)
    esrc2d_d = dp("esrc2d", [P, NT1], i32)
    dloc2d_d = dp("dloc2d", [P, NT1], f32)
    mblk2d_d = dp("mblk2d", [DB, MB], i32)
    mnode2d_d = dp("mnode2d", [P, MC // P], i32)
    esrc2m_d = dp("esrc2m", [P, NT2], i32)
    dloc2m_d = dp("dloc2m", [P, NT2], f32)

    out_t = nc.declare_dram_parameter("out_t", [2, MC], f32, isOutput=True)
    dbg_outs = {}
    if dbg:
        for nm, shp, dt_ in [("dbg_a1", [N, 16], bf16),
                             ("dbg_agg", [F * P, H1 * NACT], f32),
                             ("dbg_x2t", [48 * P, NACT], f32),
                             ("dbg_h2", [NACT, HW], f32),
                             ("dbg_o2", [MC, D], f32)]:
            dbg_outs[nm] = nc.declare_dram_parameter(nm, shp, dt_, isOutput=True)

    CORE_IDS = list(range(NCORES))

    with tile.TileContext(nc) as tc:
        with tc.tile_pool(name="const", bufs=1) as cpool, \
             tc.tile_pool(name="big", bufs=1) as bigpool, \
             tc.tile_pool(name="work", bufs=2) as wpool, \
             tc.tile_pool(name="edge", bufs=2) as epool, \
             tc.tile_pool(name="stream", bufs=3) as spool, \
             tc.tile_pool(name="dram", bufs=1, space="DRAM") as dpool:

            # ---- big streams first on the DMA queue ----
            xt_sb = cpool.tile([P, F, NSH], bf16)
            nc.sync.dma_start(out=xt_sb[:], in_=XTloc[:])
            xta_sb = cpool.tile([P, F, NACT], bf16)
            nc.sync.dma_start(out=xta_sb[:], in_=XTact[:])
            w1a_sb = cpool.tile([P, F, 16], bf16)
            nc.sync.dma_start(out=w1a_sb[:],
                              in_=w1ain[:].rearrange("p (k j) -> p k j", j=16))

            # ---- small resident tables ----
            ident = cpool.tile([P, P], f32)
            make_identity(nc, ident[:])
            identbf = cpool.tile([P, P], bf16)
            nc.vector.tensor_copy(out=identbf[:], in_=ident[:])
            onesbf = cpool.tile([P, 1], bf16)
            nc.vector.memset(onesbf[:], 1.0)
            esrc_sb = cpool.tile([P, NT1], i32)
            nc.sync.dma_start(out=esrc_sb[:], in_=esrc2d_d[:])
            esrc2m_sb = cpool.tile([P, NT2], i32)
            nc.sync.dma_start(out=esrc2m_sb[:], in_=esrc2m_d[:])
            mnode_sb = cpool.tile([P, MC // P], i32)
            nc.sync.dma_start(out=mnode_sb[:], in_=mnode2d_d[:])
            dloc_sb = cpool.tile([P, NT1], f32)
            nc.sync.dma_start(out=dloc_sb[:], in_=dloc2d_d[:])
            iotar_sb = cpool.tile([P, DB], f32)
            nc.sync.dma_start(out=iotar_sb[:], in_=iotar[:])
            att2_sb = cpool.tile([P, F, 2], bf16)
            nc.sync.dma_start(out=att2_sb[:],
                              in_=att2sb[:].rearrange("p (m j) -> p m j", j=2))
            b1_sb = cpool.tile([P, 48], f32)
            nc.sync.dma_start(out=b1_sb[:], in_=b1in[:])
            b2bc_sb = cpool.tile([P, D], f32)
            nc.sync.dma_start(out=b2bc_sb[:], in_=b2bcin[:])
            colsm_sb = cpool.tile([P, F], f32)
            nc.sync.dma_start(out=colsm_sb[:], in_=colsmin[:])
            fcb_sb = cpool.tile([P, F], f32)
            nc.sync.dma_start(out=fcb_sb[:], in_=fcbin[:])
            clsb_sb = cpool.tile([2, 1], f32)
            nc.sync.dma_start(out=clsb_sb[:], in_=clsbin[:])
            mblk_sb = cpool.tile([DB, MB], i32)
            nc.sync.dma_start(out=mblk_sb[:], in_=mblk2d_d[:])
            dloc2m_sb = cpool.tile([P, NT2], f32)
            nc.sync.dma_start(out=dloc2m_sb[:], in_=dloc2m_d[:])

            # ---- persistent big SBUF tensors ----
            h2t_sb = bigpool.tile([P, F, NACT], bf16)
            x2t_sb = bigpool.tile([P, 48, NACT], bf16)
            xem_r = bigpool.tile([P, MC // P, D], bf16)

            # ---- internal DRAM ----
            a1loc = dpool.tile([NSH, AW], bf16)
            a1_dram = dpool.tile([N, AW], bf16, addr_space="Shared")
            h2loc = dpool.tile([NACT, HW], bf16)
            a2loc = dpool.tile([NACT, 2], bf16)
            h2ag = dpool.tile([NCORES * NACT, HW], bf16, addr_space="Shared")

            with tc.tile_pool(name="aggp", bufs=1) as aggpool:
              aggT_sb = aggpool.tile([P, F, H1, NACT], fp8)
              with tc.tile_pool(name="xgp", bufs=1) as xgpool:
                # ====== gathers: X rows per edge + x_emb rows (1 call each) =
                xgall = xgpool.tile([P, NT1, D], bf16, name="xgall")
                for t in range(NT1):
                    nc.gpsimd.indirect_dma_start(
                        out=xgall[:, t, :], out_offset=None, in_=Xbf[:],
                        in_offset=bass.IndirectOffsetOnAxis(
                            ap=esrc_sb[:, t:t + 1], axis=0))
                for q in range(MC // P):
                    nc.gpsimd.indirect_dma_start(
                        out=xem_r[:, q, :], out_offset=None, in_=Xbf[:],
                        in_offset=bass.IndirectOffsetOnAxis(
                            ap=mnode_sb[:, q:q + 1], axis=0))

                # ========= S2: local a1 = X_loc @ w1a, AllGather ============
                with tc.tile_pool(name="ps2", bufs=1, space="PSUM") as ps2:
                    a1r_sb = wpool.tile([P, NSH // P, 16], bf16, tag="a1r",
                                        bufs=1)
                    for ch in range(NSH // 512):
                        a1t_ps = ps2.tile([16, 512], f32, tag="a1t", bufs=2)
                        for kt in range(F):
                            nc.tensor.matmul(
                                out=a1t_ps[:], lhsT=w1a_sb[:, kt, :],
                                rhs=xt_sb[:, kt, ch * 512:(ch + 1) * 512],
                                start=(kt == 0), stop=(kt == F - 1))
                        a1t_sb = wpool.tile([16, 512], f32, tag="a1ts")
                        nc.vector.tensor_copy(out=a1t_sb[:], in_=a1t_ps[:])
                        for q in range(4):
                            tr_ps = ps2.tile([P, 16], f32, tag="tr16", bufs=2)
                            nc.tensor.transpose(
                                out=tr_ps[:],
                                in_=a1t_sb[:16, q * P:(q + 1) * P],
                                identity=ident[:16, :16])
                            nc.vector.tensor_copy(out=a1r_sb[:, ch * 4 + q, :],
                                                  in_=tr_ps[:])
                    nc.sync.dma_start(
                        out=a1loc[:].rearrange("(a p) b -> p a b", p=P),
                        in_=a1r_sb[:])
                    # dst-side scores for active nodes straight from XTact
                    a1blk_sb = cpool.tile([DB, NB, H1], bf16)
                    for ch in range(NC_CH):
                        aat_ps = ps2.tile([H1, 512], f32, tag="aat", bufs=2)
                        for kt in range(F):
                            nc.tensor.matmul(
                                out=aat_ps[:], lhsT=w1a_sb[:, kt, 8:16],
                                rhs=xta_sb[:, kt, ch * 512:(ch + 1) * 512],
                                start=(kt == 0), stop=(kt == F - 1))
                        aat_sb = wpool.tile([H1, 512], f32, tag="aats")
                        nc.vector.tensor_copy(out=aat_sb[:], in_=aat_ps[:])
                        for qq in range(8):
                            tra_ps = ps2.tile([DB, H1], f32, tag="tra", bufs=2)
                            nc.tensor.transpose(
                                out=tra_ps[:],
                                in_=aat_sb[:H1, qq * DB:(qq + 1) * DB],
                                identity=ident[:H1, :H1])
                            nc.vector.tensor_copy(out=a1blk_sb[:, ch * 8 + qq, :],
                                                  in_=tra_ps[:])
                nc.gpsimd.collective_compute(
                    "AllGather", mybir.AluOpType.bypass,
                    replica_groups=[CORE_IDS], ins=[a1loc[:]],
                    outs=[a1_dram[:]])
                if dbg:
                    nc.sync.dma_start(out=dbg_outs["dbg_a1"][:],
                                      in_=a1_dram[:])

                # ==== S3 prep: one-hots, transposes, dst scores (all tiles,
                # independent of the a1 AllGather -> overlaps it) ============
                a01_all = xgpool.tile([P, NT1, DB], bf16, name="a01_all")
                a01t_all = xgpool.tile([DB, NT1, P], bf16, name="a01t_all")
                adb_all = xgpool.tile([P, NT1, H1], f32, name="adb_all")
                g1all = xgpool.tile([P, NT1, AW], bf16, name="g1all")
                with tc.tile_pool(name="ps3", bufs=1, space="PSUM") as ps3:
                    for b in range(NB):
                        for tt in range(TB):
                            t = b * TB + tt
                            nc.vector.tensor_scalar(
                                out=a01_all[:, t, :], in0=iotar_sb[:],
                                scalar1=dloc_sb[:, t:t + 1], scalar2=None,
                                op0=mybir.AluOpType.is_equal)
                            trA_ps = ps3.tile([DB, P], bf16, tag="trA", bufs=2)
                            nc.tensor.transpose(out=trA_ps[:],
                                                in_=a01_all[:, t, :],
                                                identity=identbf[:])
                            nc.vector.tensor_copy(out=a01t_all[:, t, :],
                                                  in_=trA_ps[:])
                            adb_ps = ps3.tile([P, H1], f32, tag="adbre", bufs=2)
                            nc.tensor.matmul(out=adb_ps[:],
                                             lhsT=a01t_all[:, t, :],
                                             rhs=a1blk_sb[:, b, :],
                                             start=True, stop=True)
                            nc.vector.tensor_copy(out=adb_all[:, t, :],
                                                  in_=adb_ps[:])

                    # a1 rows per edge (after the AllGather)
                    for t in range(NT1):
                        nc.gpsimd.indirect_dma_start(
                            out=g1all[:, t, :], out_offset=None,
                            in_=a1_dram[:],
                            in_offset=bass.IndirectOffsetOnAxis(
                                ap=esrc_sb[:, t:t + 1], axis=0))

                    # ==== batched per-edge scores: s, leaky-relu, exp =======
                    s_all = xgpool.tile([P, NT1, H1], f32, name="s_all")
                    nc.vector.tensor_tensor(out=s_all[:],
                                            in0=g1all[:, :, :H1],
                                            in1=adb_all[:],
                                            op=mybir.AluOpType.add)
                    lk_all = xgpool.tile([P, NT1, H1], f32, name="lk_all")
                    nc.vector.scalar_tensor_tensor(
                        out=lk_all[:], in0=s_all[:], scalar=NEG_SLOPE,
                        in1=s_all[:], op0=mybir.AluOpType.mult,
                        op1=mybir.AluOpType.max)
                    p1_all = xgpool.tile([P, NT1, H1], bf16, name="p1_all")
                    nc.scalar.activation(
                        out=p1_all[:], in_=lk_all[:],
                        func=mybir.ActivationFunctionType.Exp)

                    # ==== per-block: softmax denom + alpha-scaled one-hot
                    # aggregation via TensorE ================================
                    for b in range(NB):
                        dn_ps = ps3.tile([DB, H1], f32, tag="dn", bufs=1)
                        for tt in range(TB):
                            t = b * TB + tt
                            nc.tensor.matmul(out=dn_ps[:],
                                             lhsT=a01_all[:, t, :],
                                             rhs=p1_all[:, t, :],
                                             start=(tt == 0),
                                             stop=(tt == TB - 1))
                        # pad dst slots have no edges: dn=0 would give inf
                        # and 0*inf=NaN in the broadcast matmul below
                        dneps = wpool.tile([DB, H1], f32, tag="dneps")
                        nc.vector.tensor_scalar(out=dneps[:], in0=dn_ps[:],
                                                scalar1=1e-20, scalar2=None,
                                                op0=mybir.AluOpType.add)
                        recip_sb = wpool.tile([DB, H1], bf16, tag="recip")
                        with nc.allow_low_precision(
                                reason="softmax denom in bf16"):
                            nc.vector.reciprocal(out=recip_sb[:], in_=dneps[:])
                        reb_ps = ps3.tile([P, TB, H1], f32, tag="adbre",
                                          bufs=2)
                        for tt in range(TB):
                            t = b * TB + tt
                            nc.tensor.matmul(out=reb_ps[:, tt, :],
                                             lhsT=a01t_all[:, t, :],
                                             rhs=recip_sb[:],
                                             start=True, stop=True)
                        alphab = epool.tile([P, TB, H1], bf16, tag="alpha",
                                            bufs=2)
                        nc.vector.tensor_tensor(
                            out=alphab[:], in0=p1_all[:, b * TB:(b + 1) * TB, :],
                            in1=reb_ps[:], op=mybir.AluOpType.mult)
                        aalls = []
                        for tt in range(TB):
                            t = b * TB + tt
                            a_all = epool.tile([P, H1 * DB], bf16, tag="aall",
                                               bufs=TB + 1)
                            # a_all[e, h*64+d] = a01[e, d] * alpha[e, h]
                            # via stride-0 broadcast APs (one DVE op)
                            nc.vector.tensor_tensor(
                                out=a_all[:].rearrange("p (h d) -> p h d",
                                                       h=H1),
                                in0=_bcast(a01_all[:, t, :], 1, H1),
                                in1=_bcast(alphab[:, tt, :], 2, DB),
                                op=mybir.AluOpType.mult)
                            aalls.append(a_all)
                        for fs in range(2):
                            for f in range(fs * 3, fs * 3 + 3):
                                ag_ps = ps3.tile([P, H1 * DB], f32,
                                                 tag=f"agg{f % 3}", bufs=1)
                                for tt in range(TB):
                                    nc.tensor.matmul(
                                        out=ag_ps[:],
                                        lhsT=xgall[:, b * TB + tt,
                                                   f * P:(f + 1) * P],
                                        rhs=aalls[tt][:],
                                        start=(tt == 0), stop=(tt == TB - 1))
                                agdst = aggT_sb[:, f, :, b * DB:(b + 1) * DB]
                                agsrc = ag_ps[:].rearrange(
                                    "p (h d) -> p h d", h=H1)
                                if f % 2 == 1:
                                    nc.scalar.activation(
                                        out=agdst, in_=agsrc,
                                        func=mybir.ActivationFunctionType.Copy)
                                else:
                                    nc.vector.tensor_copy(out=agdst, in_=agsrc)
              # ======= S4: x2 = elu(aggT @ W1 + b1) + 1, SBUF-resident ======
              inv_s4 = 1.0 / WSCALE
              with tc.tile_pool(name="ps4", bufs=1, space="PSUM") as ps4:
                for h in range(H1):
                    for m in range(F):
                        j = h * F + m
                        w1hm = spool.tile([P, F, P], fp8, tag="w1hm", bufs=3)
                        nc.sync.dma_start(
                            out=w1hm[:],
                            in_=W1r[:].rearrange("p (j k q) -> p j k q",
                                                 j=48, k=F)[:, j, :, :])
                        for cc in range(NC_CH):
                            o1_ps = ps4.tile([P, 512], f32, tag="o1", bufs=2)
                            for k2 in range(F // 2):
                                nc.tensor.matmul(
                                    out=o1_ps[:],
                                    lhsT=w1hm[:, 2 * k2:2 * k2 + 2, :],
                                    rhs=aggT_sb[:, 2 * k2:2 * k2 + 2, h,
                                                cc * 512:(cc + 1) * 512],
                                    start=(k2 == 0), stop=(k2 == F // 2 - 1),
                                    perf_mode=mybir.MatmulPerfMode.DoubleRow)
                            ebuf = wpool.tile([P, 512], bf16, tag="ebuf")
                            nc.scalar.activation(
                                out=ebuf[:], in_=o1_ps[:],
                                func=mybir.ActivationFunctionType.Exp,
                                bias=b1_sb[:, j:j + 1], scale=inv_s4)
                            t1a = wpool.tile([P, 512], bf16, tag="t1a")
                            if j % 2 == 0:
                                nc.scalar.activation(
                                    out=t1a[:], in_=o1_ps[:],
                                    func=mybir.ActivationFunctionType.Relu,
                                    bias=b1_sb[:, j:j + 1], scale=inv_s4)
                            else:
                                xb = wpool.tile([P, 512], f32, tag="xb")
                                nc.vector.tensor_scalar(
                                    out=xb[:], in0=o1_ps[:],
                                    scalar1=inv_s4,
                                    scalar2=b1_sb[:, j:j + 1],
                                    op0=mybir.AluOpType.mult,
                                    op1=mybir.AluOpType.add)
                                nc.vector.tensor_scalar(
                                    out=t1a[:], in0=xb[:], scalar1=0.0,
                                    scalar2=None, op0=mybir.AluOpType.max)
                            # x2' = x2 + 1 = relu(xb) + min(exp(xb), 1); the +1
                            # is compensated via colsum(W2) subtraction from h2
                            nc.vector.scalar_tensor_tensor(
                                out=x2t_sb[:, j, cc * 512:(cc + 1) * 512],
                                in0=ebuf[:], scalar=1.0, in1=t1a[:],
                                op0=mybir.AluOpType.min,
                                op1=mybir.AluOpType.add)
              if dbg:
                  for f in range(F):
                      for hh in range(H1):
                          agf = wpool.tile([P, NACT], f32, tag="dbgxf")
                          nc.vector.tensor_copy(out=agf[:],
                                                in_=aggT_sb[:, f, hh, :])
                          nc.sync.dma_start(
                              out=dbg_outs["dbg_agg"]
                              [f * P:(f + 1) * P,
                               hh * NACT:(hh + 1) * NACT],
                              in_=agf[:])
            if dbg:
                for kt in range(48):
                    xxf = wpool.tile([P, NACT], f32, tag="dbgxf")
                    nc.vector.tensor_copy(out=xxf[:], in_=x2t_sb[:, kt, :])
                    nc.sync.dma_start(
                        out=dbg_outs["dbg_x2t"][kt * P:(kt + 1) * P, :],
                        in_=xxf[:])

            # ---- tail-phase SBUF tensors (after layer-1 pools are freed) ---
            with tc.tile_pool(name="tailp", bufs=1) as tailpool:
                # ========= S5: h2 = x2' @ W2 - colsum, a2, h2 rows ==========
                h2n_sb = tailpool.tile([P, NACT // P, HW], bf16, name="h2n_sb")
                with tc.tile_pool(name="ps5", bufs=1, space="PSUM") as ps5:
                    for cc in range(NC_CH):
                        h2_ps = [ps5.tile([P, 512], f32, tag=f"h2_{m}", bufs=1,
                                          name=f"h2ps{m}")
                                 for m in range(F)]
                        for kt in range(48):
                            w2kt = spool.tile([P, D], bf16, tag="w2kt", bufs=4)
                            nc.sync.dma_start(
                                out=w2kt[:],
                                in_=W2r[:].rearrange("p (k n) -> p k n",
                                                     n=D)[:, kt, :])
                            for m in range(F):
                                nc.tensor.matmul(
                                    out=h2_ps[m][:],
                                    lhsT=w2kt[:, m * P:(m + 1) * P],
                                    rhs=x2t_sb[:, kt, cc * 512:(cc + 1) * 512],
                                    start=(kt == 0), stop=(kt == 47))
                        for m in range(F):
                            nc.vector.tensor_scalar(
                                out=h2t_sb[:, m, cc * 512:(cc + 1) * 512],
                                in0=h2_ps[m][:],
                                scalar1=colsm_sb[:, m:m + 1], scalar2=None,
                                op0=mybir.AluOpType.subtract)
                        for q4 in range(4):
                            q = cc * 4 + q4
                            c0 = cc * 512 + q4 * P
                            for m in range(F):
                                trh_ps = ps5.tile([P, P], bf16, tag="trh",
                                                  bufs=1)
                                nc.tensor.transpose(
                                    out=trh_ps[:],
                                    in_=h2t_sb[:, m, c0:c0 + P],
                                    identity=identbf[:])
                                nc.vector.tensor_copy(
                                    out=h2n_sb[:, q, m * P:(m + 1) * P],
                                    in_=trh_ps[:])
                            a2_ps = ps5.tile([P, 2], f32, tag="a2", bufs=1)
                            for m in range(F):
                                nc.tensor.matmul(out=a2_ps[:],
                                                 lhsT=h2t_sb[:, m, c0:c0 + P],
                                                 rhs=att2_sb[:, m, :],
                                                 start=(m == 0),
                                                 stop=(m == F - 1))
                            a2b = wpool.tile([P, 2], bf16, tag="a2b")
                            nc.vector.tensor_copy(out=a2b[:], in_=a2_ps[:])
                            nc.vector.tensor_copy(out=h2n_sb[:, q, D:D + 2],
                                                  in_=a2b[:])
                            nc.vector.tensor_copy(
                                out=h2n_sb[:, q, D + 2:D + 3], in_=onesbf[:])
                            nc.sync.dma_start(out=a2loc[q * P:(q + 1) * P, :],
                                              in_=a2b[:])
                nc.sync.dma_start(
                    out=h2loc[:].rearrange("(a p) b -> p a b", p=P),
                    in_=h2n_sb[:])
                if dbg:
                    for q in range(NACT // P):
                        h2f = wpool.tile([P, HW], f32, tag="dbgxf2")
                        nc.vector.tensor_copy(out=h2f[:], in_=h2n_sb[:, q, :])
                        nc.sync.dma_start(
                            out=dbg_outs["dbg_h2"][q * P:(q + 1) * P, :],
                            in_=h2f[:])

                # a2 dst rows of the masked slots (gathered by active slot)
                a2blk_sb = cpool.tile([DB, MB, 2], bf16)
                for b2 in range(MB):
                    nc.gpsimd.indirect_dma_start(
                        out=a2blk_sb[:, b2, :], out_offset=None, in_=a2loc[:],
                        in_offset=bass.IndirectOffsetOnAxis(
                            ap=mblk_sb[:, b2:b2 + 1], axis=0))

                # ===== S7 prep + S8 prep (overlap the h2 AllGather) =========
                with tc.tile_pool(name="ps8", bufs=1, space="PSUM") as ps8:
                    # x_emb transposes -> feature-major
                    xem_bf = tailpool.tile([P, F, MC], bf16)
                    for q in range(MC // P):
                        for f in range(F):
                            tre_ps = ps8.tile([P, P], bf16, tag="tr", bufs=2)
                            nc.tensor.transpose(
                                out=tre_ps[:],
                                in_=xem_r[:, q, f * P:(f + 1) * P],
                                identity=identbf[:])
                            nc.vector.tensor_copy(
                                out=xem_bf[:, f, q * P:(q + 1) * P],
                                in_=tre_ps[:])
                    # fc weights + classifier weights
                    fcw_sb = tailpool.tile([P, F, 12, P], bf16)
                    for m in range(F):
                        nc.sync.dma_start(
                            out=fcw_sb[:, m, :, :],
                            in_=fcwr[:].rearrange("p (m k q) -> p m k q",
                                                  m=F, k=12)[:, m, :, :])
                    clsw_sb = cpool.tile([P, F, 2], bf16)
                    nc.sync.dma_start(
                        out=clsw_sb[:],
                        in_=clswbf[:].rearrange("(m p) n -> p m n", p=P))
                    # x_emb half of fc, accumulated to SBUF (psum banks are
                    # needed by S7 while the collective runs)
                    fcacc = tailpool.tile([P, F, MC], f32, name="fcacc")
                    for m in range(F):
                        fcA_ps = ps8.tile([P, MC], f32, tag="fcA", bufs=2)
                        for kt in range(6, 12):
                            nc.tensor.matmul(out=fcA_ps[:],
                                             lhsT=fcw_sb[:, m, kt, :],
                                             rhs=xem_bf[:, kt - F, :],
                                             start=(kt == 6), stop=(kt == 11))
                        nc.vector.tensor_scalar(out=fcacc[:, m, :],
                                                in0=fcA_ps[:],
                                                scalar1=fcb_sb[:, m:m + 1],
                                                scalar2=None,
                                                op0=mybir.AluOpType.add)

                    # layer-2 one-hots + dst scores (independent of collective)
                    a01m = tailpool.tile([P, NT2, DB], bf16, name="a01m")
                    a01tm = tailpool.tile([DB, NT2, P], bf16, name="a01tm")
                    ad2b = tailpool.tile([P, NT2, 1], f32, name="ad2b")
                    for b2 in range(MB):
                        for tt in range(TB2):
                            t = b2 * TB2 + tt
                            nc.vector.tensor_scalar(
                                out=a01m[:, t, :], in0=iotar_sb[:],
                                scalar1=dloc2m_sb[:, t:t + 1], scalar2=None,
                                op0=mybir.AluOpType.is_equal)
                            trm_ps = ps8.tile([DB, P], bf16, tag="tr", bufs=2)
                            nc.tensor.transpose(out=trm_ps[:],
                                                in_=a01m[:, t, :],
                                                identity=identbf[:])
                            nc.vector.tensor_copy(out=a01tm[:, t, :],
                                                  in_=trm_ps[:])
                            ad2_ps = ps8.tile([P, 1], f32, tag="tr", bufs=2)
                            nc.tensor.matmul(out=ad2_ps[:],
                                             lhsT=a01tm[:, t, :],
                                             rhs=a2blk_sb[:, b2, 1:2],
                                             start=True, stop=True)
                            nc.vector.tensor_copy(out=ad2b[:, t, :],
                                                  in_=ad2_ps[:])

                    # ========= S6: AllGather h2 rows ========================
                    nc.gpsimd.collective_compute(
                        "AllGather", mybir.AluOpType.bypass,
                        replica_groups=[CORE_IDS], ins=[h2loc[:]],
                        outs=[h2ag[:]])

                    # h2 rows per layer-2 edge
                    hgall = tailpool.tile([P, NT2, HW], bf16, name="hgall")
                    for t in range(NT2):
                        nc.gpsimd.indirect_dma_start(
                            out=hgall[:, t, :], out_offset=None, in_=h2ag[:],
                            in_offset=bass.IndirectOffsetOnAxis(
                                ap=esrc2m_sb[:, t:t + 1], axis=0))

                    # batched layer-2 scores
                    s2_all = tailpool.tile([P, NT2, 1], f32, name="s2_all")
                    nc.vector.tensor_tensor(out=s2_all[:],
                                            in0=hgall[:, :, D:D + 1],
                                            in1=ad2b[:],
                                            op=mybir.AluOpType.add)
                    lk2_all = tailpool.tile([P, NT2, 1], f32, name="lk2_all")
                    nc.vector.scalar_tensor_tensor(
                        out=lk2_all[:], in0=s2_all[:], scalar=NEG_SLOPE,
                        in1=s2_all[:], op0=mybir.AluOpType.mult,
                        op1=mybir.AluOpType.max)
                    p2_all = tailpool.tile([P, NT2, 1], f32, name="p2_all")
                    nc.scalar.activation(out=p2_all[:], in_=lk2_all[:],
                                         func=mybir.ActivationFunctionType.Exp)

                    # ========= S7: layer-2 edge phase, masked dst only ======
                    xgm_bf = tailpool.tile([P, F, MC], bf16)
                    for b2 in range(MB):
                        outA_ps = ps8.tile([DB, 512], f32, tag="outA", bufs=1)
                        outB_ps = ps8.tile([DB, HW - 512], f32, tag="outB",
                                           bufs=1)
                        for tt in range(TB2):
                            t = b2 * TB2 + tt
                            a_all = epool.tile([P, DB], bf16, tag="aall2")
                            if tt % 2 == 0:
                                nc.vector.tensor_scalar(
                                    out=a_all[:], in0=a01m[:, t, :],
                                    scalar1=p2_all[:, t, 0:1], scalar2=None,
                                    op0=mybir.AluOpType.mult)
                            else:
                                nc.scalar.activation(
                                    out=a_all[:], in_=a01m[:, t, :],
                                    func=mybir.ActivationFunctionType.Copy,
                                    scale=p2_all[:, t, 0:1])
                            nc.tensor.matmul(out=outA_ps[:], lhsT=a_all[:],
                                             rhs=hgall[:, t, 0:512],
                                             start=(tt == 0),
                                             stop=(tt == TB2 - 1))
                            nc.tensor.matmul(out=outB_ps[:], lhsT=a_all[:],
                                             rhs=hgall[:, t, 512:HW],
                                             start=(tt == 0),
                                             stop=(tt == TB2 - 1))
                        recd_sb = wpool.tile([DB, 1], f32, tag="recd")
                        nc.vector.reciprocal(
                            out=recd_sb[:],
                            in_=outB_ps[:, D + 2 - 512:D + 3 - 512])
                        o2_sb = wpool.tile([DB, D], f32, tag="o2sb")
                        nc.vector.scalar_tensor_tensor(
                            out=o2_sb[:, 0:512], in0=outA_ps[:],
                            scalar=recd_sb[:, 0:1], in1=b2bc_sb[:DB, 0:512],
                            op0=mybir.AluOpType.mult, op1=mybir.AluOpType.add)
                        nc.vector.scalar_tensor_tensor(
                            out=o2_sb[:, 512:D], in0=outB_ps[:, 0:D - 512],
                            scalar=recd_sb[:, 0:1], in1=b2bc_sb[:DB, 512:D],
                            op0=mybir.AluOpType.mult, op1=mybir.AluOpType.add)
                        if dbg:
                            nc.sync.dma_start(
                                out=dbg_outs["dbg_o2"][b2 * DB:(b2 + 1) * DB, :],
                                in_=o2_sb[:])
                        for f in range(F):
                            tro_ps = ps8.tile([P, DB], f32, tag="tr", bufs=2)
                            nc.tensor.transpose(
                                out=tro_ps[:],
                                in_=o2_sb[:, f * P:(f + 1) * P],
                                identity=ident[:DB, :DB])
                            nc.vector.tensor_copy(
                                out=xgm_bf[:, f, b2 * DB:(b2 + 1) * DB],
                                in_=tro_ps[:])

                    # ===== S8: finish fc (x_gemb half) + classifier =========
                    fcT_bf = tailpool.tile([P, F, MC], bf16)
                    for m in range(F):
                        fcB_ps = ps8.tile([P, MC], f32, tag="fcA", bufs=2)
                        for kt in range(6):
                            nc.tensor.matmul(out=fcB_ps[:],
                                             lhsT=fcw_sb[:, m, kt, :],
                                             rhs=xgm_bf[:, kt, :],
                                             start=(kt == 0), stop=(kt == 5))
                        nc.vector.tensor_tensor(out=fcT_bf[:, m, :],
                                                in0=fcB_ps[:],
                                                in1=fcacc[:, m, :],
                                                op=mybir.AluOpType.add)
                    cls_ps = ps8.tile([2, MC], f32, tag="cls", bufs=1)
                    for m in range(F):
                        nc.tensor.matmul(out=cls_ps[:], lhsT=clsw_sb[:, m, :],
                                         rhs=fcT_bf[:, m, :],
                                         start=(m == 0), stop=(m == F - 1))
                    outf = wpool.tile([2, MC], f32, tag="outf")
                    nc.vector.tensor_scalar(out=outf[:], in0=cls_ps[:],
                                            scalar1=clsb_sb[:, 0:1],
                                            scalar2=None,
                                            op0=mybir.AluOpType.add)
                    nc.sync.dma_start(out=out_t[:], in_=outf[:])

    _split_excess_waits(nc)
    return nc


# ---------------------------------------------------------------------------
def kernel(cls_embeddings, edge_index, mask_idx, W1, att_src1, att_dst1, b1,
           W2, att_src2, att_dst2, b2, fc_w, fc_b, cls_w, cls_b, _dbg=False):
    X = np.asarray(cls_embeddings, dtype=np.float32)
    per_core, positions, act_lists, NACT, TB, MC, TB2 = _preprocess(
        np.asarray(edge_index), np.asarray(mask_idx))

    # host-folded attention basis: w1a[d, j] = sum_c W1[d, hc] att_j[h, c]
    W1f = np.asarray(W1, np.float32).reshape(D, H1, D)
    w1a = np.concatenate(
        [np.einsum("dhc,hc->dh", W1f, np.asarray(att_src1, np.float32)),
         np.einsum("dhc,hc->dh", W1f, np.asarray(att_dst1, np.float32))],
        axis=1)                                    # [768, 16]
    w1ain = np.ascontiguousarray(
        w1a.reshape(F, P, 16).transpose(1, 0, 2).reshape(P, F * 16))

    att2T = np.stack([np.asarray(att_src2, np.float32)[0],
                      np.asarray(att_dst2, np.float32)[0]], axis=1)
    att2_sb = np.ascontiguousarray(
        att2T.reshape(F, P, 2).transpose(1, 0, 2).reshape(P, F * 2))

    XT = np.ascontiguousarray(X.T).astype(BF)      # [768, 8192]

    W1h = np.asarray(W1, np.float32)
    W2h = np.asarray(W2, np.float32)
    # W1r[p, j, kt, q] = W1[kt*128+p, j*128+q] * WSCALE  (fp8)
    W1r = np.ascontiguousarray(
        (W1h * WSCALE).reshape(F, P, 48, P).transpose(1, 2, 0, 3)
        .reshape(P, 48 * F * P)).astype(F8)
    # W2r[p, kt, n] = W2[kt*128+p, n]
    W2r = np.ascontiguousarray(
        W2h.reshape(48, P, D).transpose(1, 0, 2).reshape(P, 48 * D)).astype(BF)
    # fcwr[p, m, kt, q] = fc_w[kt*128+p, m*128+q]
    fch = np.asarray(fc_w, np.float32)
    fcwr = np.ascontiguousarray(
        fch.reshape(12, P, F, P).transpose(1, 2, 0, 3)
        .reshape(P, F * 12 * P)).astype(BF)

    shared = {
        "Xbf": X.astype(BF),
        "w1ain": w1ain.astype(BF),
        "W1r": W1r,
        "W2r": W2r,
        "att2sb": att2_sb.astype(BF),
        "fcwr": fcwr,
        "clswbf": np.asarray(cls_w, np.float32).astype(BF),
        "b1in": np.ascontiguousarray(np.asarray(b1, np.float32).reshape(48, P).T),
        "b2bcin": np.tile(np.asarray(b2, np.float32).reshape(1, D), (P, 1)),
        "colsmin": np.ascontiguousarray(
            W2h.sum(axis=0, dtype=np.float64).astype(np.float32)
            .reshape(F, P).T),
        "fcbin": np.ascontiguousarray(np.asarray(fc_b, np.float32).reshape(F, P).T),
        "clsbin": np.asarray(cls_b, np.float32).reshape(2, 1),
        "iotar": np.tile(np.arange(DB, dtype=np.float32), (P, 1)),
    }

    nc = _build_program(NACT, TB, MC, TB2, dbg=_dbg)
    in_maps = []
    for c in range(NCORES):
        m = dict(shared)
        m.update(per_core[c])
        xtl = XT[:, c * NSH:(c + 1) * NSH]
        m["XTloc"] = np.ascontiguousarray(
            xtl.reshape(F, P, NSH).transpose(1, 0, 2))
        xta = np.zeros((D, NACT), dtype=BF)
        xta[:, :len(act_lists[c])] = XT[:, act_lists[c]]
        m["XTact"] = np.ascontiguousarray(
            xta.reshape(F, P, NACT).transpose(1, 0, 2))
        in_maps.append(m)

    global LAST
    kres = run_bass_kernel_spmd(nc, in_maps, list(range(NCORES)),
                                trace=TRACE, tmpdir=TRACE_DIR)
    LAST = kres
    res = kres.results

    out = np.zeros((M, 2), dtype=np.float32)
    for c in range(NCORES):
        pos = positions[c]
        ot = res[c]["out_t"]
        for j, p_ in enumerate(pos):
            out[p_] = ot[:, j]
    if _dbg:
        return out, res, positions, act_lists
    return out


# revision 17
# speedup vs baseline: 2.1132x; 1.0338x over previous
"""Two-layer GAT (PyG-style GATConv) on 8 Trainium2 NeuronCores.

v4 layout (active-set, SBUF-resident intermediates, dma_gather):
- Only "active" nodes (srcs of edges into masked dsts, plus masked dsts)
  need layer-1 output / layer-2 features: ~45% of all nodes. Layer-1
  aggregation, S4 (agg@W1), S5 (x2@W2) and the h2 exchange are restricted
  to the per-core active set (padded to NACT=512 vs 1024 owned nodes).
- Single S3 pass over all dst blocks -> aggT (fp8) fully SBUF-resident;
  S4 streams W1 once, keeps x2t in SBUF (no DRAM round-trip); S5 streams
  W2 once against the SBUF-resident x2t.
- All row gathers (X rows per edge, a1 rows per edge, h2 rows per
  layer-2 edge, x_emb rows) are single dma_gather instructions (SWDGE
  descriptor generation is ~1us + 0.34ns/row; per-tile indirect DMAs
  were the previous bottleneck). a1/h2 rows are padded to 256B multiples
  to satisfy the gather's element-size constraint.
- S3 one-hot/transpose/dst-score prep for all tiles is hoisted before the
  a1 AllGather; per-edge scores are computed in 3 batched ops; the
  alpha-scaled one-hot uses a single broadcast tensor_tensor per tile.
- h2 AllGather carries only active rows ([NACT, 896] per core); S8 prep
  (x_emb transposes, fc weight loads, fc partial accumulation) overlaps
  the collective.
- Masked fc/classifier on the owning core; host reassembles the output.
"""
import numpy as np
from concourse import bass, mybir
import concourse.tile as tile
from concourse.bass_utils import run_bass_kernel_spmd
from concourse.vector_clock import ScopedClock, VectorClock
from concourse.masks import make_identity

N, E, M = 8192, 32768, 1024
D = 768
F = 6               # 768 / 128
H1 = 8
NCORES = 8
NSH = N // NCORES   # 1024 nodes owned per core
DB = 64
P = 128
HW = D + 4          # h2 row: 768 h2 | a_s2 | a_d2 | 1.0 | pad
AW = 16             # a1 row: 8 a_src | 8 a_dst

f32 = mybir.dt.float32
bf16 = mybir.dt.bfloat16
fp8 = mybir.dt.float8e4
i32 = mybir.dt.int32
i16 = mybir.dt.int16
BF = mybir.dt.np(bf16)
F8 = mybir.dt.np(fp8)
WSCALE = 32.0       # fp8 W1 pre-scale (undone via activation scale)

NEG_SLOPE = 0.2

# test-harness knobs (harness calls kernel() with defaults: no tracing)
TRACE = False
TRACE_DIR = None
LAST = None


# ---------------------------------------------------------------------------
# The walrus build in this container rejects a Drain instruction with more
# than one semaphore wait ("Too many sync wait commands"); the default
# TileContext kernel-tail drain has many. Emit one single-wait drain per
# logical processor instead.
def _split_drain_and_barrier(self, tick_clock, wait_clock):
    gc = tick_clock.global_clock
    nprocs = 27
    for i in range(nprocs):
        mask = [0] * nprocs
        mask[i] = 1 << 30
        part = gc.elementwise_min(VectorClock(mask))
        d = self.nc.sync.drain()
        wait_clock.add_sem_waits(d.ins, ScopedClock({None: part}))
    self.nc.all_engine_barrier()
    popped = self.nc._tile_sem_poison_stack.pop()
    assert popped is self._sem_poison
    self.nc.clear_and_free_semaphores(list(self.sems.allocated().values()))
    self.nc.all_engine_barrier()


tile.TileContext._drain_and_barrier = _split_drain_and_barrier

MAX_WAITS = 1  # this walrus build rejects multi-sem-wait instructions


def _split_excess_waits(nc):
    """Move excess semaphore waits onto preceding same-engine NoOps."""
    n_split = 0
    for bb in nc.m.functions[0].blocks:
        insts = bb.instructions
        idx = 0
        while idx < len(insts):
            inst = insts[idx]
            si = inst.sync_info
            if si is not None and len(si.on_wait) > MAX_WAITS:
                waits = list(si.on_wait)
                keep = waits[-MAX_WAITS:]
                extra = waits[:-MAX_WAITS]
                for gi in range(0, len(extra), MAX_WAITS):
                    nop = mybir.InstNoOp(
                        name=f"WSPLIT-{nc.next_id()}",
                        sync_info=mybir.SyncInfo(
                            on_wait=extra[gi:gi + MAX_WAITS], on_update=[]),
                        bass_nofuse=True,
                        engine=inst.engine,
                        ins=[], outs=[],
                    )
                    nc.register_instruction(nop)
                    insts.insert(idx, nop)
                    idx += 1
                inst.sync_info = mybir.SyncInfo(
                    on_wait=keep, on_update=list(si.on_update))
                n_split += 1
            idx += 1
    return n_split


def _bcast(ap, pos, n):
    """Insert a stride-0 broadcast dim of size n at free-dim position pos."""
    layout = [list(d) for d in ap.ap]
    layout.insert(pos, [0, n])
    return bass.AP(ap.tensor, ap.offset, layout)


def _wrap16(flat):
    """dma_gather index table: [128, n/16] i16, idx i at [i%16, i//16],
    replicated across the 8 Q7 cores (partition groups of 16)."""
    t = np.asarray(flat, dtype=np.int16).reshape(-1, 16).T
    return np.ascontiguousarray(np.tile(t, (8, 1)))


# ---------------------------------------------------------------------------
def _preprocess(edge_index, mask_idx):
    """Host-side graph partitioning: integer index work only."""
    src = np.asarray(edge_index[0], dtype=np.int64)
    dst = np.asarray(edge_index[1], dtype=np.int64)
    loop = np.arange(N, dtype=np.int64)
    src = np.concatenate([src, loop])
    dst = np.concatenate([dst, loop])
    mask = np.asarray(mask_idx, dtype=np.int64)

    # active set: masked nodes + srcs of edges into masked nodes
    mset = np.zeros(N, dtype=bool)
    mset[mask] = True
    need = np.zeros(N, dtype=bool)
    need[src[mset[dst]]] = True
    need[mask] = True

    aslot = np.full(N, -1, dtype=np.int64)
    act_lists = []
    for c in range(NCORES):
        nodes = np.nonzero(need[c * NSH:(c + 1) * NSH])[0] + c * NSH
        act_lists.append(nodes)
        aslot[nodes] = np.arange(len(nodes))
    NACT = int(np.ceil(max(len(a) for a in act_lists) / 512)) * 512
    NB = NACT // DB
    gid = (np.arange(N) // NSH) * NACT + aslot   # row in h2ag (valid if need)

    # ---- layer-1 edges: those into active dsts, bucketed (core, block) ----
    keep = need[dst]
    e_src, e_dst = src[keep], dst[keep]
    e_core = e_dst // NSH
    e_slot = aslot[e_dst]
    bucket = e_core * NB + e_slot // DB
    counts = np.bincount(bucket, minlength=NCORES * NB)
    TB = int(np.ceil(counts.max() / P))
    NT1 = NB * TB
    order = np.argsort(bucket, kind='stable')
    starts = np.zeros(NCORES * NB + 1, dtype=np.int64)
    np.cumsum(counts, out=starts[1:])
    pos = np.arange(len(order)) - starts[bucket[order]]
    flat = np.zeros((NCORES, NB * TB * P), dtype=np.int64)      # src node ids
    dflat = np.full((NCORES, NB * TB * P), 1000.0, dtype=np.float32)
    bo = bucket[order]
    addr = (bo % NB) * TB * P + pos
    flat[e_core[order], addr] = e_src[order]
    dflat[e_core[order], addr] = (e_slot[order] % DB).astype(np.float32)

    def to2d(a, nt):
        return np.ascontiguousarray(a.reshape(nt, P).T)

    per_core = []
    for c in range(NCORES):
        per_core.append(dict(
            esrc2d=to2d(flat[c].astype(np.int32), NT1),
            dloc2d=to2d(dflat[c], NT1),
        ))

    # ---- masked nodes per owning core ----
    positions = [np.nonzero(mask // NSH == c)[0] for c in range(NCORES)]
    MC = max(P, int(np.ceil(max(len(p) for p in positions) / P)) * P)
    MB = MC // DB

    # layer-2 edges grouped per masked occurrence (src is always active)
    l2 = mset[dst]
    l2_src, l2_dst = src[l2], dst[l2]
    d_order = np.argsort(l2_dst, kind='stable')
    sd = l2_dst[d_order]
    cnt2 = np.zeros((NCORES, MB), dtype=np.int64)
    for c in range(NCORES):
        pos_c = positions[c]
        nodes = mask[pos_c]
        lo = np.searchsorted(sd, nodes, side='left')
        hi = np.searchsorted(sd, nodes, side='right')
        deg = hi - lo
        nb2 = np.minimum(np.arange(len(pos_c)) // DB, MB - 1)
        np.add.at(cnt2[c], nb2, deg)
    TB2 = int(np.ceil(cnt2.max() / P))
    NT2 = MB * TB2
    for c in range(NCORES):
        pos_c = positions[c]
        nodes = mask[pos_c]
        mnode = np.zeros(MC, dtype=np.int64)
        mnode[:len(pos_c)] = nodes
        mloc = np.zeros(MC, dtype=np.int64)
        mloc[:len(pos_c)] = aslot[nodes]
        per_core[c]["mnode2d"] = np.ascontiguousarray(
            mnode.reshape(MC // P, P).T.astype(np.int32))
        per_core[c]["mblk2d"] = np.ascontiguousarray(
            mloc.reshape(MB, DB).T.astype(np.int32))
        esrc2 = np.zeros(NT2 * P, dtype=np.int64)
        dloc2 = np.full(NT2 * P, 1000.0, dtype=np.float32)
        lo = np.searchsorted(sd, nodes, side='left')
        hi = np.searchsorted(sd, nodes, side='right')
        for b2 in range(MB):
            base = b2 * TB2 * P
            k = 0
            for j in range(b2 * DB, min((b2 + 1) * DB, len(pos_c))):
                for e in d_order[lo[j]:hi[j]]:
                    esrc2[base + k] = gid[l2_src[e]]
                    dloc2[base + k] = j % DB
                    k += 1
        per_core[c]["esrc2m"] = to2d(esrc2.astype(np.int32), NT2)
        per_core[c]["dloc2m"] = to2d(dloc2, NT2)
    return per_core, positions, act_lists, NACT, TB, MC, TB2


# ---------------------------------------------------------------------------
def _build_program(NACT, TB, MC, TB2, b1_zero=True, dbg=False):
    NB = NACT // DB
    NC_CH = NACT // 512          # 512-wide chunks of the active set
    MB = MC // DB
    NT1 = NB * TB
    NT2 = MB * TB2
    nc = bass.Bass("TRN2", target_bir_lowering=False, debug=False,
                   num_devices=NCORES)
    dp = lambda name, shape, dt: nc.declare_dram_parameter(
        name, list(shape), dt, isOutput=False)

    Xbf = dp("Xbf", [N, D], bf16)
    XTloc = dp("XTloc", [P, F, NSH], bf16)           # per-core X.T (own nodes)
    XTact = dp("XTact", [P, F, NACT], bf16)          # per-core X.T (active)
    w1ain = dp("w1ain", [P, F * 16], bf16)           # host-folded att basis
    W1r = dp("W1r", [P, 48 * F * P], fp8)            # [p, j, kt, q] fp8*WSCALE
    W2r = dp("W2r", [P, 48 * D], bf16)               # [p, kt, n]
    att2sb = dp("att2sb", [P, F * 2], bf16)
    fcwr = dp("fcwr", [P, F * 12 * P], bf16)         # [p, m, kt, q]
    clswbf = dp("clswbf", [D, 2], bf16)
    b1in = dp("b1in", [P, 48], f32)
    b2bcin = dp("b2bcin", [P, D], f32)
    colsmin = dp("colsmin", [P, F], f32)
    fcbin = dp("fcbin", [P, F], f32)
    clsbin = dp("clsbin", [2, 1], f32)
    iotar = dp("iotar", [P, DB], f32)
    esrc2d_d = dp("esrc2d", [P, NT1], i32)
    dloc2d_d = dp("dloc2d", [P, NT1], f32)
    mblk2d_d = dp("mblk2d", [DB, MB], i32)
    mnode2d_d = dp("mnode2d", [P, MC // P], i32)
    esrc2m_d = dp("esrc2m", [P, NT2], i32)
    dloc2m_d = dp("dloc2m", [P, NT2], f32)

    out_t = nc.declare_dram_parameter("out_t", [2, MC], f32, isOutput=True)
    dbg_outs = {}
    if dbg:
        for nm, shp, dt_ in [("dbg_a1", [N, 16], bf16),
                             ("dbg_agg", [F * P, H1 * NACT], f32),
                             ("dbg_x2t", [48 * P, NACT], f32),
                             ("dbg_h2", [NACT, HW], f32),
                             ("dbg_o2", [MC, D], f32)]:
            dbg_outs[nm] = nc.declare_dram_parameter(nm, shp, dt_, isOutput=True)

    CORE_IDS = list(range(NCORES))

    with tile.TileContext(nc) as tc:
        with tc.tile_pool(name="const", bufs=1) as cpool, \
             tc.tile_pool(name="big", bufs=1) as bigpool, \
             tc.tile_pool(name="work", bufs=2) as wpool, \
             tc.tile_pool(name="edge", bufs=2) as epool, \
             tc.tile_pool(name="stream", bufs=3) as spool, \
             tc.tile_pool(name="dram", bufs=1, space="DRAM") as dpool:

            # ---- big streams first on the DMA queue ----
            xt_sb = cpool.tile([P, F, NSH], bf16)
            nc.sync.dma_start(out=xt_sb[:], in_=XTloc[:])
            xta_sb = cpool.tile([P, F, NACT], bf16)
            nc.sync.dma_start(out=xta_sb[:], in_=XTact[:])
            w1a_sb = cpool.tile([P, F, 16], bf16)
            nc.sync.dma_start(out=w1a_sb[:],
                              in_=w1ain[:].rearrange("p (k j) -> p k j", j=16))

            # ---- small resident tables ----
            ident = cpool.tile([P, P], f32)
            make_identity(nc, ident[:])
            identbf = cpool.tile([P, P], bf16)
            nc.vector.tensor_copy(out=identbf[:], in_=ident[:])
            onesbf = cpool.tile([P, 1], bf16)
            nc.vector.memset(onesbf[:], 1.0)
            esrc_sb = cpool.tile([P, NT1], i32)
            nc.sync.dma_start(out=esrc_sb[:], in_=esrc2d_d[:])
            esrc2m_sb = cpool.tile([P, NT2], i32)
            nc.sync.dma_start(out=esrc2m_sb[:], in_=esrc2m_d[:])
            mnode_sb = cpool.tile([P, MC // P], i32)
            nc.sync.dma_start(out=mnode_sb[:], in_=mnode2d_d[:])
            dloc_sb = cpool.tile([P, NT1], f32)
            nc.sync.dma_start(out=dloc_sb[:], in_=dloc2d_d[:])
            iotar_sb = cpool.tile([P, DB], f32)
            nc.sync.dma_start(out=iotar_sb[:], in_=iotar[:])
            att2_sb = cpool.tile([P, F, 2], bf16)
            nc.sync.dma_start(out=att2_sb[:],
                              in_=att2sb[:].rearrange("p (m j) -> p m j", j=2))
            b1_sb = cpool.tile([P, 48], f32)
            nc.sync.dma_start(out=b1_sb[:], in_=b1in[:])
            b2bc_sb = cpool.tile([P, D], f32)
            nc.sync.dma_start(out=b2bc_sb[:], in_=b2bcin[:])
            colsm_sb = cpool.tile([P, F], f32)
            nc.sync.dma_start(out=colsm_sb[:], in_=colsmin[:])
            fcb_sb = cpool.tile([P, F], f32)
            nc.sync.dma_start(out=fcb_sb[:], in_=fcbin[:])
            clsb_sb = cpool.tile([2, 1], f32)
            nc.sync.dma_start(out=clsb_sb[:], in_=clsbin[:])
            mblk_sb = cpool.tile([DB, MB], i32)
            nc.sync.dma_start(out=mblk_sb[:], in_=mblk2d_d[:])
            dloc2m_sb = cpool.tile([P, NT2], f32)
            nc.sync.dma_start(out=dloc2m_sb[:], in_=dloc2m_d[:])

            # ---- persistent big SBUF tensors ----
            h2t_sb = bigpool.tile([P, F, NACT], bf16)
            x2t_sb = bigpool.tile([P, 48, NACT], bf16)
            xem_r = bigpool.tile([P, MC // P, D], bf16)

            # ---- internal DRAM ----
            a1loc = dpool.tile([NSH, AW], bf16)
            a1_dram = dpool.tile([N, AW], bf16, addr_space="Shared")
            h2loc = dpool.tile([NACT, HW], bf16)
            a2loc = dpool.tile([NACT, 2], bf16)
            h2ag = dpool.tile([NCORES * NACT, HW], bf16, addr_space="Shared")

            with tc.tile_pool(name="aggp", bufs=1) as aggpool:
              aggT_sb = aggpool.tile([P, F, H1, NACT], fp8)
              with tc.tile_pool(name="xgp", bufs=1) as xgpool:
                # ====== gathers: X rows per edge + x_emb rows (1 call each) =
                xgall = xgpool.tile([P, NT1, D], bf16, name="xgall")
                for t in range(NT1):
                    nc.gpsimd.indirect_dma_start(
                        out=xgall[:, t, :], out_offset=None, in_=Xbf[:],
                        in_offset=bass.IndirectOffsetOnAxis(
                            ap=esrc_sb[:, t:t + 1], axis=0))
                for q in range(MC // P):
                    nc.gpsimd.indirect_dma_start(
                        out=xem_r[:, q, :], out_offset=None, in_=Xbf[:],
                        in_offset=bass.IndirectOffsetOnAxis(
                            ap=mnode_sb[:, q:q + 1], axis=0))

                # ========= S2: local a1 = X_loc @ w1a, AllGather ============
                with tc.tile_pool(name="ps2", bufs=1, space="PSUM") as ps2:
                    a1r_sb = wpool.tile([P, NSH // P, 16], bf16, tag="a1r",
                                        bufs=1)
                    for ch in range(NSH // 512):
                        a1t_ps = ps2.tile([16, 512], f32, tag="a1t", bufs=2)
                        for kt in range(F):
                            nc.tensor.matmul(
                                out=a1t_ps[:], lhsT=w1a_sb[:, kt, :],
                                rhs=xt_sb[:, kt, ch * 512:(ch + 1) * 512],
                                start=(kt == 0), stop=(kt == F - 1))
                        a1t_sb = wpool.tile([16, 512], f32, tag="a1ts")
                        nc.vector.tensor_copy(out=a1t_sb[:], in_=a1t_ps[:])
                        for q in range(4):
                            tr_ps = ps2.tile([P, 16], f32, tag="tr16", bufs=2)
                            nc.tensor.transpose(
                                out=tr_ps[:],
                                in_=a1t_sb[:16, q * P:(q + 1) * P],
                                identity=ident[:16, :16])
                            nc.vector.tensor_copy(out=a1r_sb[:, ch * 4 + q, :],
                                                  in_=tr_ps[:])
                    nc.sync.dma_start(
                        out=a1loc[:].rearrange("(a p) b -> p a b", p=P),
                        in_=a1r_sb[:])
                    # dst-side scores for active nodes straight from XTact
                    a1blk_sb = cpool.tile([DB, NB, H1], bf16)
                    for ch in range(NC_CH):
                        aat_ps = ps2.tile([H1, 512], f32, tag="aat", bufs=2)
                        for kt in range(F):
                            nc.tensor.matmul(
                                out=aat_ps[:], lhsT=w1a_sb[:, kt, 8:16],
                                rhs=xta_sb[:, kt, ch * 512:(ch + 1) * 512],
                                start=(kt == 0), stop=(kt == F - 1))
                        aat_sb = wpool.tile([H1, 512], f32, tag="aats")
                        nc.vector.tensor_copy(out=aat_sb[:], in_=aat_ps[:])
                        for qq in range(8):
                            tra_ps = ps2.tile([DB, H1], f32, tag="tra", bufs=2)
                            nc.tensor.transpose(
                                out=tra_ps[:],
                                in_=aat_sb[:H1, qq * DB:(qq + 1) * DB],
                                identity=ident[:H1, :H1])
                            nc.vector.tensor_copy(out=a1blk_sb[:, ch * 8 + qq, :],
                                                  in_=tra_ps[:])
                nc.gpsimd.collective_compute(
                    "AllGather", mybir.AluOpType.bypass,
                    replica_groups=[CORE_IDS], ins=[a1loc[:]],
                    outs=[a1_dram[:]])
                if dbg:
                    nc.sync.dma_start(out=dbg_outs["dbg_a1"][:],
                                      in_=a1_dram[:])

                # ==== S3 prep: one-hots, transposes, dst scores (all tiles,
                # independent of the a1 AllGather -> overlaps it) ============
                a01_all = xgpool.tile([P, NT1, DB], bf16, name="a01_all")
                a01t_all = xgpool.tile([DB, NT1, P], bf16, name="a01t_all")
                adb_all = xgpool.tile([P, NT1, H1], f32, name="adb_all")
                g1all = xgpool.tile([P, NT1, AW], bf16, name="g1all")
                with tc.tile_pool(name="ps3", bufs=1, space="PSUM") as ps3:
                    for b in range(NB):
                        for tt in range(TB):
                            t = b * TB + tt
                            nc.vector.tensor_scalar(
                                out=a01_all[:, t, :], in0=iotar_sb[:],
                                scalar1=dloc_sb[:, t:t + 1], scalar2=None,
                                op0=mybir.AluOpType.is_equal)
                            trA_ps = ps3.tile([DB, P], bf16, tag="trA", bufs=2)
                            nc.tensor.transpose(out=trA_ps[:],
                                                in_=a01_all[:, t, :],
                                                identity=identbf[:])
                            nc.vector.tensor_copy(out=a01t_all[:, t, :],
                                                  in_=trA_ps[:])
                            adb_ps = ps3.tile([P, H1], f32, tag="adbre", bufs=2)
                            nc.tensor.matmul(out=adb_ps[:],
                                             lhsT=a01t_all[:, t, :],
                                             rhs=a1blk_sb[:, b, :],
                                             start=True, stop=True)
                            nc.vector.tensor_copy(out=adb_all[:, t, :],
                                                  in_=adb_ps[:])

                    # ==== per-block: gather a1 rows, per-edge scores,
                    # softmax denom + alpha-scaled one-hot aggregation =======
                    for b in range(NB):
                        for tt in range(TB):
                            t = b * TB + tt
                            nc.gpsimd.indirect_dma_start(
                                out=g1all[:, t, :], out_offset=None,
                                in_=a1_dram[:],
                                in_offset=bass.IndirectOffsetOnAxis(
                                    ap=esrc_sb[:, t:t + 1], axis=0))
                        s_sb = epool.tile([P, TB, H1], f32, tag="s", bufs=2)
                        nc.vector.tensor_tensor(
                            out=s_sb[:],
                            in0=g1all[:, b * TB:(b + 1) * TB, :H1],
                            in1=adb_all[:, b * TB:(b + 1) * TB, :],
                            op=mybir.AluOpType.add)
                        lk_sb = epool.tile([P, TB, H1], f32, tag="lk", bufs=2)
                        nc.vector.scalar_tensor_tensor(
                            out=lk_sb[:], in0=s_sb[:], scalar=NEG_SLOPE,
                            in1=s_sb[:], op0=mybir.AluOpType.mult,
                            op1=mybir.AluOpType.max)
                        p1_sb = epool.tile([P, TB, H1], bf16, tag="p1", bufs=2)
                        nc.scalar.activation(
                            out=p1_sb[:], in_=lk_sb[:],
                            func=mybir.ActivationFunctionType.Exp)
                        dn_ps = ps3.tile([DB, H1], f32, tag="dn", bufs=1)
                        for tt in range(TB):
                            t = b * TB + tt
                            nc.tensor.matmul(out=dn_ps[:],
                                             lhsT=a01_all[:, t, :],
                                             rhs=p1_sb[:, tt, :],
                                             start=(tt == 0),
                                             stop=(tt == TB - 1))
                        # pad dst slots have no edges: dn=0 would give inf
                        # and 0*inf=NaN in the broadcast matmul below
                        dneps = wpool.tile([DB, H1], f32, tag="dneps")
                        nc.vector.tensor_scalar(out=dneps[:], in0=dn_ps[:],
                                                scalar1=1e-20, scalar2=None,
                                                op0=mybir.AluOpType.add)
                        recip_sb = wpool.tile([DB, H1], bf16, tag="recip")
                        with nc.allow_low_precision(
                                reason="softmax denom in bf16"):
                            nc.vector.reciprocal(out=recip_sb[:], in_=dneps[:])
                        reb_ps = ps3.tile([P, TB, H1], f32, tag="adbre",
                                          bufs=2)
                        for tt in range(TB):
                            t = b * TB + tt
                            nc.tensor.matmul(out=reb_ps[:, tt, :],
                                             lhsT=a01t_all[:, t, :],
                                             rhs=recip_sb[:],
                                             start=True, stop=True)
                        alphab = epool.tile([P, TB, H1], bf16, tag="alpha",
                                            bufs=2)
                        nc.vector.tensor_tensor(
                            out=alphab[:], in0=p1_sb[:],
                            in1=reb_ps[:], op=mybir.AluOpType.mult)
                        aalls = []
                        for tt in range(TB):
                            t = b * TB + tt
                            a_all = epool.tile([P, H1 * DB], bf16, tag="aall",
                                               bufs=TB + 1)
                            # a_all[e, h*64+d] = a01[e, d] * alpha[e, h]
                            # via stride-0 broadcast APs (one DVE op)
                            nc.vector.tensor_tensor(
                                out=a_all[:].rearrange("p (h d) -> p h d",
                                                       h=H1),
                                in0=_bcast(a01_all[:, t, :], 1, H1),
                                in1=_bcast(alphab[:, tt, :], 2, DB),
                                op=mybir.AluOpType.mult)
                            aalls.append(a_all)
                        for fs in range(2):
                            for f in range(fs * 3, fs * 3 + 3):
                                ag_ps = ps3.tile([P, H1 * DB], f32,
                                                 tag=f"agg{f % 3}", bufs=1)
                                for tt in range(TB):
                                    nc.tensor.matmul(
                                        out=ag_ps[:],
                                        lhsT=xgall[:, b * TB + tt,
                                                   f * P:(f + 1) * P],
                                        rhs=aalls[tt][:],
                                        start=(tt == 0), stop=(tt == TB - 1))
                                agdst = aggT_sb[:, f, :, b * DB:(b + 1) * DB]
                                agsrc = ag_ps[:].rearrange(
                                    "p (h d) -> p h d", h=H1)
                                if f % 2 == 1:
                                    nc.scalar.activation(
                                        out=agdst, in_=agsrc,
                                        func=mybir.ActivationFunctionType.Copy)
                                else:
                                    nc.vector.tensor_copy(out=agdst, in_=agsrc)
              # ======= S4: x2 = elu(aggT @ W1 + b1) + 1, SBUF-resident ======
              inv_s4 = 1.0 / WSCALE
              with tc.tile_pool(name="ps4", bufs=1, space="PSUM") as ps4:
                for h in range(H1):
                    for m in range(F):
                        j = h * F + m
                        w1hm = spool.tile([P, F, P], fp8, tag="w1hm", bufs=3)
                        nc.sync.dma_start(
                            out=w1hm[:],
                            in_=W1r[:].rearrange("p (j k q) -> p j k q",
                                                 j=48, k=F)[:, j, :, :])
                        for cc in range(NC_CH):
                            o1_ps = ps4.tile([P, 512], f32, tag="o1", bufs=2)
                            for k2 in range(F // 2):
                                nc.tensor.matmul(
                                    out=o1_ps[:],
                                    lhsT=w1hm[:, 2 * k2:2 * k2 + 2, :],
                                    rhs=aggT_sb[:, 2 * k2:2 * k2 + 2, h,
                                                cc * 512:(cc + 1) * 512],
                                    start=(k2 == 0), stop=(k2 == F // 2 - 1),
                                    perf_mode=mybir.MatmulPerfMode.DoubleRow)
                            ebuf = wpool.tile([P, 512], bf16, tag="ebuf")
                            nc.scalar.activation(
                                out=ebuf[:], in_=o1_ps[:],
                                func=mybir.ActivationFunctionType.Exp,
                                bias=b1_sb[:, j:j + 1], scale=inv_s4)
                            t1a = wpool.tile([P, 512], bf16, tag="t1a")
                            if j % 2 == 0:
                                nc.scalar.activation(
                                    out=t1a[:], in_=o1_ps[:],
                                    func=mybir.ActivationFunctionType.Relu,
                                    bias=b1_sb[:, j:j + 1], scale=inv_s4)
                            elif b1_zero:
                                # relu(inv*o1 + 0) == relu(o1) * inv
                                nc.vector.tensor_scalar(
                                    out=t1a[:], in0=o1_ps[:], scalar1=0.0,
                                    scalar2=inv_s4,
                                    op0=mybir.AluOpType.max,
                                    op1=mybir.AluOpType.mult)
                            else:
                                xb = wpool.tile([P, 512], f32, tag="xb")
                                nc.vector.tensor_scalar(
                                    out=xb[:], in0=o1_ps[:],
                                    scalar1=inv_s4,
                                    scalar2=b1_sb[:, j:j + 1],
                                    op0=mybir.AluOpType.mult,
                                    op1=mybir.AluOpType.add)
                                nc.vector.tensor_scalar(
                                    out=t1a[:], in0=xb[:], scalar1=0.0,
                                    scalar2=None, op0=mybir.AluOpType.max)
                            # x2' = x2 + 1 = relu(xb) + min(exp(xb), 1); the +1
                            # is compensated via colsum(W2) subtraction from h2
                            nc.vector.scalar_tensor_tensor(
                                out=x2t_sb[:, j, cc * 512:(cc + 1) * 512],
                                in0=ebuf[:], scalar=1.0, in1=t1a[:],
                                op0=mybir.AluOpType.min,
                                op1=mybir.AluOpType.add)
              if dbg:
                  for f in range(F):
                      for hh in range(H1):
                          agf = wpool.tile([P, NACT], f32, tag="dbgxf")
                          nc.vector.tensor_copy(out=agf[:],
                                                in_=aggT_sb[:, f, hh, :])
                          nc.sync.dma_start(
                              out=dbg_outs["dbg_agg"]
                              [f * P:(f + 1) * P,
                               hh * NACT:(hh + 1) * NACT],
                              in_=agf[:])
            if dbg:
                for kt in range(48):
                    xxf = wpool.tile([P, NACT], f32, tag="dbgxf")
                    nc.vector.tensor_copy(out=xxf[:], in_=x2t_sb[:, kt, :])
                    nc.sync.dma_start(
                        out=dbg_outs["dbg_x2t"][kt * P:(kt + 1) * P, :],
                        in_=xxf[:])

            # ---- tail-phase SBUF tensors (after layer-1 pools are freed) ---
            with tc.tile_pool(name="tailp", bufs=1) as tailpool:
                # ========= S5: h2 = x2' @ W2 - colsum, a2, h2 rows ==========
                h2n_sb = tailpool.tile([P, NACT // P, HW], bf16, name="h2n_sb")
                with tc.tile_pool(name="ps5", bufs=1, space="PSUM") as ps5:
                    for cc in range(NC_CH):
                        h2_ps = [ps5.tile([P, 512], f32, tag=f"h2_{m}", bufs=1,
                                          name=f"h2ps{m}")
                                 for m in range(F)]
                        for kt in range(48):
                            w2kt = spool.tile([P, D], bf16, tag="w2kt", bufs=4)
                            nc.sync.dma_start(
                                out=w2kt[:],
                                in_=W2r[:].rearrange("p (k n) -> p k n",
                                                     n=D)[:, kt, :])
                            for m in range(F):
                                nc.tensor.matmul(
                                    out=h2_ps[m][:],
                                    lhsT=w2kt[:, m * P:(m + 1) * P],
                                    rhs=x2t_sb[:, kt, cc * 512:(cc + 1) * 512],
                                    start=(kt == 0), stop=(kt == 47))
                        for m in range(F):
                            nc.vector.tensor_scalar(
                                out=h2t_sb[:, m, cc * 512:(cc + 1) * 512],
                                in0=h2_ps[m][:],
                                scalar1=colsm_sb[:, m:m + 1], scalar2=None,
                                op0=mybir.AluOpType.subtract)
                        for q4 in range(4):
                            q = cc * 4 + q4
                            c0 = cc * 512 + q4 * P
                            for m in range(F):
                                trh_ps = ps5.tile([P, P], bf16, tag="trh",
                                                  bufs=1)
                                nc.tensor.transpose(
                                    out=trh_ps[:],
                                    in_=h2t_sb[:, m, c0:c0 + P],
                                    identity=identbf[:])
                                nc.vector.tensor_copy(
                                    out=h2n_sb[:, q, m * P:(m + 1) * P],
                                    in_=trh_ps[:])
                            a2_ps = ps5.tile([P, 2], f32, tag="a2", bufs=1)
                            for m in range(F):
                                nc.tensor.matmul(out=a2_ps[:],
                                                 lhsT=h2t_sb[:, m, c0:c0 + P],
                                                 rhs=att2_sb[:, m, :],
                                                 start=(m == 0),
                                                 stop=(m == F - 1))
                            a2b = wpool.tile([P, 2], bf16, tag="a2b")
                            nc.vector.tensor_copy(out=a2b[:], in_=a2_ps[:])
                            nc.vector.tensor_copy(out=h2n_sb[:, q, D:D + 2],
                                                  in_=a2b[:])
                            nc.vector.tensor_copy(
                                out=h2n_sb[:, q, D + 2:D + 3], in_=onesbf[:])
                            nc.sync.dma_start(out=a2loc[q * P:(q + 1) * P, :],
                                              in_=a2b[:])
                nc.sync.dma_start(
                    out=h2loc[:].rearrange("(a p) b -> p a b", p=P),
                    in_=h2n_sb[:])
                if dbg:
                    for q in range(NACT // P):
                        h2f = wpool.tile([P, HW], f32, tag="dbgxf2")
                        nc.vector.tensor_copy(out=h2f[:], in_=h2n_sb[:, q, :])
                        nc.sync.dma_start(
                            out=dbg_outs["dbg_h2"][q * P:(q + 1) * P, :],
                            in_=h2f[:])

                # a2 dst rows of the masked slots (gathered by active slot)
                a2blk_sb = cpool.tile([DB, MB, 2], bf16)
                for b2 in range(MB):
                    nc.gpsimd.indirect_dma_start(
                        out=a2blk_sb[:, b2, :], out_offset=None, in_=a2loc[:],
                        in_offset=bass.IndirectOffsetOnAxis(
                            ap=mblk_sb[:, b2:b2 + 1], axis=0))

                # ===== S7 prep + S8 prep (overlap the h2 AllGather) =========
                with tc.tile_pool(name="ps8", bufs=1, space="PSUM") as ps8:
                    # x_emb transposes -> feature-major
                    xem_bf = tailpool.tile([P, F, MC], bf16)
                    for q in range(MC // P):
                        for f in range(F):
                            tre_ps = ps8.tile([P, P], bf16, tag="tr", bufs=2)
                            nc.tensor.transpose(
                                out=tre_ps[:],
                                in_=xem_r[:, q, f * P:(f + 1) * P],
                                identity=identbf[:])
                            nc.vector.tensor_copy(
                                out=xem_bf[:, f, q * P:(q + 1) * P],
                                in_=tre_ps[:])
                    # fc weights + classifier weights
                    fcw_sb = tailpool.tile([P, F, 12, P], bf16)
                    for m in range(F):
                        nc.sync.dma_start(
                            out=fcw_sb[:, m, :, :],
                            in_=fcwr[:].rearrange("p (m k q) -> p m k q",
                                                  m=F, k=12)[:, m, :, :])
                    clsw_sb = cpool.tile([P, F, 2], bf16)
                    nc.sync.dma_start(
                        out=clsw_sb[:],
                        in_=clswbf[:].rearrange("(m p) n -> p m n", p=P))
                    # x_emb half of fc, accumulated to SBUF (psum banks are
                    # needed by S7 while the collective runs)
                    fcacc = tailpool.tile([P, F, MC], f32, name="fcacc")
                    for m in range(F):
                        fcA_ps = ps8.tile([P, MC], f32, tag="fcA", bufs=2)
                        for kt in range(6, 12):
                            nc.tensor.matmul(out=fcA_ps[:],
                                             lhsT=fcw_sb[:, m, kt, :],
                                             rhs=xem_bf[:, kt - F, :],
                                             start=(kt == 6), stop=(kt == 11))
                        nc.vector.tensor_scalar(out=fcacc[:, m, :],
                                                in0=fcA_ps[:],
                                                scalar1=fcb_sb[:, m:m + 1],
                                                scalar2=None,
                                                op0=mybir.AluOpType.add)

                    # layer-2 one-hots + dst scores (independent of collective)
                    a01m = tailpool.tile([P, NT2, DB], bf16, name="a01m")
                    a01tm = tailpool.tile([DB, NT2, P], bf16, name="a01tm")
                    ad2b = tailpool.tile([P, NT2, 1], f32, name="ad2b")
                    for b2 in range(MB):
                        for tt in range(TB2):
                            t = b2 * TB2 + tt
                            nc.vector.tensor_scalar(
                                out=a01m[:, t, :], in0=iotar_sb[:],
                                scalar1=dloc2m_sb[:, t:t + 1], scalar2=None,
                                op0=mybir.AluOpType.is_equal)
                            trm_ps = ps8.tile([DB, P], bf16, tag="tr", bufs=2)
                            nc.tensor.transpose(out=trm_ps[:],
                                                in_=a01m[:, t, :],
                                                identity=identbf[:])
                            nc.vector.tensor_copy(out=a01tm[:, t, :],
                                                  in_=trm_ps[:])
                            ad2_ps = ps8.tile([P, 1], f32, tag="tr", bufs=2)
                            nc.tensor.matmul(out=ad2_ps[:],
                                             lhsT=a01tm[:, t, :],
                                             rhs=a2blk_sb[:, b2, 1:2],
                                             start=True, stop=True)
                            nc.vector.tensor_copy(out=ad2b[:, t, :],
                                                  in_=ad2_ps[:])

                    # ========= S6: AllGather h2 rows ========================
                    nc.gpsimd.collective_compute(
                        "AllGather", mybir.AluOpType.bypass,
                        replica_groups=[CORE_IDS], ins=[h2loc[:]],
                        outs=[h2ag[:]])

                    # ========= S7: layer-2 edge phase, masked dst only ======
                    hgall = tailpool.tile([P, NT2, HW], bf16, name="hgall")
                    xgm_bf = tailpool.tile([P, F, MC], bf16)
                    for b2 in range(MB):
                        for tt in range(TB2):
                            t = b2 * TB2 + tt
                            nc.gpsimd.indirect_dma_start(
                                out=hgall[:, t, :], out_offset=None,
                                in_=h2ag[:],
                                in_offset=bass.IndirectOffsetOnAxis(
                                    ap=esrc2m_sb[:, t:t + 1], axis=0))
                        s2_sb = epool.tile([P, TB2, 1], f32, tag="s2", bufs=2)
                        nc.vector.tensor_tensor(
                            out=s2_sb[:],
                            in0=hgall[:, b2 * TB2:(b2 + 1) * TB2, D:D + 1],
                            in1=ad2b[:, b2 * TB2:(b2 + 1) * TB2, :],
                            op=mybir.AluOpType.add)
                        lk2_sb = epool.tile([P, TB2, 1], f32, tag="lk2",
                                            bufs=2)
                        nc.vector.scalar_tensor_tensor(
                            out=lk2_sb[:], in0=s2_sb[:], scalar=NEG_SLOPE,
                            in1=s2_sb[:], op0=mybir.AluOpType.mult,
                            op1=mybir.AluOpType.max)
                        p2_sb = epool.tile([P, TB2, 1], f32, tag="p2", bufs=2)
                        nc.scalar.activation(
                            out=p2_sb[:], in_=lk2_sb[:],
                            func=mybir.ActivationFunctionType.Exp)
                        outA_ps = ps8.tile([DB, 512], f32, tag="outA", bufs=1)
                        outB_ps = ps8.tile([DB, HW - 512], f32, tag="outB",
                                           bufs=1)
                        for tt in range(TB2):
                            t = b2 * TB2 + tt
                            a_all = epool.tile([P, DB], bf16, tag="aall2")
                            if tt % 2 == 0:
                                nc.vector.tensor_scalar(
                                    out=a_all[:], in0=a01m[:, t, :],
                                    scalar1=p2_sb[:, tt, 0:1], scalar2=None,
                                    op0=mybir.AluOpType.mult)
                            else:
                                nc.scalar.activation(
                                    out=a_all[:], in_=a01m[:, t, :],
                                    func=mybir.ActivationFunctionType.Copy,
                                    scale=p2_sb[:, tt, 0:1])
                            nc.tensor.matmul(out=outA_ps[:], lhsT=a_all[:],
                                             rhs=hgall[:, t, 0:512],
                                             start=(tt == 0),
                                             stop=(tt == TB2 - 1))
                            nc.tensor.matmul(out=outB_ps[:], lhsT=a_all[:],
                                             rhs=hgall[:, t, 512:HW],
                                             start=(tt == 0),
                                             stop=(tt == TB2 - 1))
                        recd_sb = wpool.tile([DB, 1], f32, tag="recd")
                        nc.vector.reciprocal(
                            out=recd_sb[:],
                            in_=outB_ps[:, D + 2 - 512:D + 3 - 512])
                        o2_sb = wpool.tile([DB, D], f32, tag="o2sb")
                        nc.vector.scalar_tensor_tensor(
                            out=o2_sb[:, 0:512], in0=outA_ps[:],
                            scalar=recd_sb[:, 0:1], in1=b2bc_sb[:DB, 0:512],
                            op0=mybir.AluOpType.mult, op1=mybir.AluOpType.add)
                        nc.vector.scalar_tensor_tensor(
                            out=o2_sb[:, 512:D], in0=outB_ps[:, 0:D - 512],
                            scalar=recd_sb[:, 0:1], in1=b2bc_sb[:DB, 512:D],
                            op0=mybir.AluOpType.mult, op1=mybir.AluOpType.add)
                        if dbg:
                            nc.sync.dma_start(
                                out=dbg_outs["dbg_o2"][b2 * DB:(b2 + 1) * DB, :],
                                in_=o2_sb[:])
                        for f in range(F):
                            tro_ps = ps8.tile([P, DB], f32, tag="tr", bufs=2)
                            nc.tensor.transpose(
                                out=tro_ps[:],
                                in_=o2_sb[:, f * P:(f + 1) * P],
                                identity=ident[:DB, :DB])
                            nc.vector.tensor_copy(
                                out=xgm_bf[:, f, b2 * DB:(b2 + 1) * DB],
                                in_=tro_ps[:])

                    # ===== S8: finish fc (x_gemb half) + classifier =========
                    fcT_bf = tailpool.tile([P, F, MC], bf16)
                    for m in range(F):
                        fcB_ps = ps8.tile([P, MC], f32, tag="fcA", bufs=2)
                        for kt in range(6):
                            nc.tensor.matmul(out=fcB_ps[:],
                                             lhsT=fcw_sb[:, m, kt, :],
                                             rhs=xgm_bf[:, kt, :],
                                             start=(kt == 0), stop=(kt == 5))
                        nc.vector.tensor_tensor(out=fcT_bf[:, m, :],
                                                in0=fcB_ps[:],
                                                in1=fcacc[:, m, :],
                                                op=mybir.AluOpType.add)
                    cls_ps = ps8.tile([2, MC], f32, tag="cls", bufs=1)
                    for m in range(F):
                        nc.tensor.matmul(out=cls_ps[:], lhsT=clsw_sb[:, m, :],
                                         rhs=fcT_bf[:, m, :],
                                         start=(m == 0), stop=(m == F - 1))
                    outf = wpool.tile([2, MC], f32, tag="outf")
                    nc.vector.tensor_scalar(out=outf[:], in0=cls_ps[:],
                                            scalar1=clsb_sb[:, 0:1],
                                            scalar2=None,
                                            op0=mybir.AluOpType.add)
                    nc.sync.dma_start(out=out_t[:], in_=outf[:])

    _split_excess_waits(nc)
    return nc


# ---------------------------------------------------------------------------
def kernel(cls_embeddings, edge_index, mask_idx, W1, att_src1, att_dst1, b1,
           W2, att_src2, att_dst2, b2, fc_w, fc_b, cls_w, cls_b, _dbg=False):
    X = np.asarray(cls_embeddings, dtype=np.float32)
    per_core, positions, act_lists, NACT, TB, MC, TB2 = _preprocess(
        np.asarray(edge_index), np.asarray(mask_idx))

    # host-folded attention basis: w1a[d, j] = sum_c W1[d, hc] att_j[h, c]
    W1f = np.asarray(W1, np.float32).reshape(D, H1, D)
    w1a = np.concatenate(
        [np.einsum("dhc,hc->dh", W1f, np.asarray(att_src1, np.float32)),
         np.einsum("dhc,hc->dh", W1f, np.asarray(att_dst1, np.float32))],
        axis=1)                                    # [768, 16]
    w1ain = np.ascontiguousarray(
        w1a.reshape(F, P, 16).transpose(1, 0, 2).reshape(P, F * 16))

    att2T = np.stack([np.asarray(att_src2, np.float32)[0],
                      np.asarray(att_dst2, np.float32)[0]], axis=1)
    att2_sb = np.ascontiguousarray(
        att2T.reshape(F, P, 2).transpose(1, 0, 2).reshape(P, F * 2))

    XT = np.ascontiguousarray(X.T).astype(BF)      # [768, 8192]

    W1h = np.asarray(W1, np.float32)
    W2h = np.asarray(W2, np.float32)
    # W1r[p, j, kt, q] = W1[kt*128+p, j*128+q] * WSCALE  (fp8)
    W1r = np.ascontiguousarray(
        (W1h * WSCALE).reshape(F, P, 48, P).transpose(1, 2, 0, 3)
        .reshape(P, 48 * F * P)).astype(F8)
    # W2r[p, kt, n] = W2[kt*128+p, n]
    W2r = np.ascontiguousarray(
        W2h.reshape(48, P, D).transpose(1, 0, 2).reshape(P, 48 * D)).astype(BF)
    # fcwr[p, m, kt, q] = fc_w[kt*128+p, m*128+q]
    fch = np.asarray(fc_w, np.float32)
    fcwr = np.ascontiguousarray(
        fch.reshape(12, P, F, P).transpose(1, 2, 0, 3)
        .reshape(P, F * 12 * P)).astype(BF)

    shared = {
        "Xbf": X.astype(BF),
        "w1ain": w1ain.astype(BF),
        "W1r": W1r,
        "W2r": W2r,
        "att2sb": att2_sb.astype(BF),
        "fcwr": fcwr,
        "clswbf": np.asarray(cls_w, np.float32).astype(BF),
        "b1in": np.ascontiguousarray(np.asarray(b1, np.float32).reshape(48, P).T),
        "b2bcin": np.tile(np.asarray(b2, np.float32).reshape(1, D), (P, 1)),
        "colsmin": np.ascontiguousarray(
            W2h.sum(axis=0, dtype=np.float64).astype(np.float32)
            .reshape(F, P).T),
        "fcbin": np.ascontiguousarray(np.asarray(fc_b, np.float32).reshape(F, P).T),
        "clsbin": np.asarray(cls_b, np.float32).reshape(2, 1),
        "iotar": np.tile(np.arange(DB, dtype=np.float32), (P, 1)),
    }

    b1_zero = bool(np.all(np.asarray(b1) == 0.0))
    nc = _build_program(NACT, TB, MC, TB2, b1_zero=b1_zero, dbg=_dbg)
    in_maps = []
    for c in range(NCORES):
        m = dict(shared)
        m.update(per_core[c])
        xtl = XT[:, c * NSH:(c + 1) * NSH]
        m["XTloc"] = np.ascontiguousarray(
            xtl.reshape(F, P, NSH).transpose(1, 0, 2))
        xta = np.zeros((D, NACT), dtype=BF)
        xta[:, :len(act_lists[c])] = XT[:, act_lists[c]]
        m["XTact"] = np.ascontiguousarray(
            xta.reshape(F, P, NACT).transpose(1, 0, 2))
        in_maps.append(m)

    global LAST
    kres = run_bass_kernel_spmd(nc, in_maps, list(range(NCORES)),
                                trace=TRACE, tmpdir=TRACE_DIR)
    LAST = kres
    res = kres.results

    out = np.zeros((M, 2), dtype=np.float32)
    for c in range(NCORES):
        pos = positions[c]
        ot = res[c]["out_t"]
        for j, p_ in enumerate(pos):
            out[p_] = ot[:, j]
    if _dbg:
        return out, res, positions, act_lists
    return out
